# revision 1
# baseline (speedup 1.0000x reference)
"""Trainium2 Bass kernel for nn_DecoderLayer (Performer/FAVOR+ decoder layer).

v2: folded-projection FAVOR+. Because the Performer uses only M=8 random
features per head (HM = H*M = 128 total), the Q/K projections fold into the
random-feature projection on the host: wqe = Wq @ P^T/sqrt(M) is [D, 128], so
qp = relu(x @ wqe) + stab needs no D x D matmul. The V and O projections fold
through the kv statistic: kv = kp^T V = (kp^T X) Wv and the output
o @ Wo = qps @ ((kv blockdiag-masked) @ Wo), so the only D x D work left is
applied to the tiny [D x 128] ST statistic instead of the full sequence.

Sharding: sequence (L) split across 8 cores; the per-batch global statistics
ST = [X^T kp | kp-sums] (~2MB total for B=4) are AllReduced twice.
Residual stream is feature-major; the final LN3 runs token-major so the
output DMAs out in natural [T, D] layout (no host post-transpose).
"""
import sys
import os

sys.path.insert(0, '/opt/trn_rl_repo')

import numpy as np
import ml_dtypes
from contextlib import ExitStack

from concourse import bass, bacc, tile
import concourse.mybir as mybir
from concourse.bass_utils import run_bass_kernel_spmd
from concourse.alu_op_type import AluOpType

F32 = mybir.dt.float32
F32R = mybir.dt.float32r
BF16 = mybir.dt.bfloat16
AF = mybir.ActivationFunctionType
AX = mybir.AxisListType

B, L, D, H, DH, M, DFF = 4, 4096, 1024, 16, 64, 8, 4096
NCORES = 8
LSH = L // NCORES          # 512 tokens of L per core
T = B * LSH                # 2048 tokens per core
TB = LSH                   # tokens per batch element per core (512)
NTS = TB // 128            # 4 x 128-token blocks per batch
DC = D // 128              # 8 d-chunks
DFC = DFF // 128           # 32 dff-chunks
HM = H * M                 # 128 random features total
EPS_LN = 1e-6
STAB = 0.001
ARW = DC * 512 + 4         # allreduce width: ST (8 chunks x 4 batches x 128) + ksums

_cache = {}


def _mm(nc, out, lhsT, rhs, start, stop, skip=False):
    nc.tensor.matmul(out, lhsT.bitcast(F32R), rhs.bitcast(F32R),
                     start=start, stop=stop, skip_group_check=skip)


def _mmb(nc, out, lhsT, rhs, start, stop, skip=False):
    nc.tensor.matmul(out, lhsT, rhs, start=start, stop=stop,
                     skip_group_check=skip)


def build_program(loop=1):
    nc = bacc.Bacc("TRN2", target_bir_lowering=False, debug=False,
                   num_devices=NCORES)

    def din(name, shape, dt=F32R):
        return nc.dram_tensor(name, shape, dt, kind="ExternalInput").ap()

    tensors = dict(
        xT=din("xT", [D, T]),
        encT=din("encT", [D, T]),
        wqe1=din("wqe1", [D, HM]), wke1=din("wke1", [D, HM]),
        wqe2=din("wqe2", [D, HM]), wke2=din("wke2", [D, HM]),
        wv1=din("wv1", [D, D], BF16), wo1=din("wo1", [D, D], BF16),
        wv2=din("wv2", [D, D], BF16), wo2=din("wo2", [D, D], BF16),
        w1=din("w1", [D, DFF], BF16), w2=din("w2", [DFF, D], BF16),
        kvm4=din("kvm4", [D, HM], BF16),
        e16T_d=din("e16T", [HM, H]), e16_d=din("e16", [H, HM]),
        gbe_d=din("gbe", [128, 4 * DC], F32),
        b1c_d=din("b1c", [128, DFC], F32),
        b2r_d=din("b2r", [1, D]),
        g3r_d=din("g3r", [1, D]), be3r_d=din("be3r", [1, D]),
        ident_d=din("ident", [128, 128]),
        identb_d=din("identb", [128, 128], BF16),
        ones_col_d=din("ones_col", [128, 8]),
        ones_row_d=din("ones_row", [1, 128]),
    )

    out_d = nc.dram_tensor("out", [T, D], F32, kind="ExternalOutput").ap()

    with nc.allow_low_precision(reason="f32r/bf16 matmul inputs"), \
         tile.TileContext(nc) as tc, ExitStack() as top:
        dram = top.enter_context(tc.tile_pool(name="dram", bufs=1, space="DRAM"))

        const = top.enter_context(tc.tile_pool(name="const", bufs=1))
        c = {}
        c['e16T'] = const.tile([HM, H], F32R, name="e16T")
        nc.sync.dma_start(c['e16T'][:], tensors['e16T_d'][:])
        c['e16'] = const.tile([H, HM], F32R, name="e16")
        nc.sync.dma_start(c['e16'][:], tensors['e16_d'][:])
        c['gbe'] = const.tile([128, 4 * DC], F32, name="gbe")
        nc.sync.dma_start(c['gbe'][:], tensors['gbe_d'][:])
        c['b1c'] = const.tile([128, DFC], F32, name="b1c")
        nc.sync.dma_start(c['b1c'][:], tensors['b1c_d'][:])
        c['b2r'] = const.tile([1, D], F32R, name="b2r")
        nc.sync.dma_start(c['b2r'][:], tensors['b2r_d'][:])
        c['ident'] = const.tile([128, 128], F32R, name="ident")
        nc.sync.dma_start(c['ident'][:], tensors['ident_d'][:])
        c['identb'] = const.tile([128, 128], BF16, name="identb")
        nc.sync.dma_start(c['identb'][:], tensors['identb_d'][:])
        c['ones_col'] = const.tile([128, 8], F32R, name="ones_col")
        nc.sync.dma_start(c['ones_col'][:], tensors['ones_col_d'][:])
        c['ones_row'] = const.tile([1, 128], F32R, name="ones_row")
        nc.sync.dma_start(c['ones_row'][:], tensors['ones_row_d'][:])
        c['eps_t'] = const.tile([1, 1], F32, name="eps_t")
        nc.vector.memset(c['eps_t'][:], EPS_LN)
        c['eps_c'] = const.tile([128, 1], F32, name="eps_c")
        nc.vector.memset(c['eps_c'][:], EPS_LN)
        g3r = const.tile([1, D], F32R, name="g3r")
        nc.sync.dma_start(g3r[:], tensors['g3r_d'][:])
        be3r = const.tile([1, D], F32R, name="be3r")
        nc.sync.dma_start(be3r[:], tensors['be3r_d'][:])
        c['g3bc'] = const.tile([128, D], F32R, name="g3bc")
        c['be3bc'] = const.tile([128, D], F32R, name="be3bc")
        with tc.tile_pool(name="bc_ps", bufs=2, space="PSUM") as bcp:
            for src, dst in ((g3r, c['g3bc']), (be3r, c['be3bc'])):
                for half in range(2):
                    pg = bcp.tile([128, D // 2], F32, tag="bc", name="pbc")
                    _mm(nc, pg[:], c['ones_row'][:],
                        src[:, half * 512:(half + 1) * 512], True, True)
                    nc.any.tensor_copy(dst[:, half * 512:(half + 1) * 512], pg[:])

        for it in range(loop):
            build_iter(nc, tc, f"i{it}" if loop > 1 else "", tensors, c,
                       dram, out_d)

    nc.compile()
    return nc


def build_iter(nc, tc, sfx, tensors, c, dram, out_d):
    e16T, e16 = c['e16T'], c['e16']
    gbe, b1c, b2r = c['gbe'], c['b1c'], c['b2r']
    ident, identb = c['ident'], c['identb']
    ones_col, ones_row = c['ones_col'], c['ones_row']
    eps_t, g3bc, be3bc = c['eps_t'], c['g3bc'], c['be3bc']
    eps_c = c['eps_c']

    arin1 = dram.tile([128, ARW], F32, name=f"arin1{sfx}")
    arout1 = dram.tile([128, ARW], F32, addr_space="Shared", name=f"arout1{sfx}")
    arin2 = dram.tile([128, ARW], F32, name=f"arin2{sfx}")
    arout2 = dram.tile([128, ARW], F32, addr_space="Shared", name=f"arout2{sfx}")
    h_spill = dram.tile([DFF, T], BF16, name=f"hspill{sfx}")

    def gslice(i):
        return gbe[:, 2 * i * DC:(2 * i + 1) * DC]

    def beslice(i):
        return gbe[:, (2 * i + 1) * DC:(2 * i + 2) * DC]

    def load_wide(pool, src_dram, ncols, name, dt=F32R):
        nchunk = src_dram.shape[0] // 128
        t_ = pool.tile([128, nchunk * ncols], dt, name=name)
        for kc in range(nchunk):
            nc.sync.dma_start(t_[:, kc * ncols:(kc + 1) * ncols],
                              src_dram[kc * 128:(kc + 1) * 128, :])
        return t_

    def feat_phase(inp_b, wqe, wke, qp_fm, arin, suffix):
        """qp/kp features + ST statistic + ksum -> arin (DRAM)."""
        with ExitStack() as ph:
            work = ph.enter_context(tc.tile_pool(name=f"ftw{suffix}", bufs=2))
            kptp = ph.enter_context(tc.tile_pool(name=f"kptp{suffix}", bufs=2))
            arp = ph.enter_context(tc.tile_pool(name=f"arp{suffix}", bufs=1))
            pqk = ph.enter_context(tc.tile_pool(name=f"pqk{suffix}", bufs=2, space="PSUM"))
            tpp = ph.enter_context(tc.tile_pool(name=f"tpp{suffix}", bufs=4, space="PSUM"))
            stps = ph.enter_context(tc.tile_pool(name=f"stps{suffix}", bufs=2, space="PSUM"))

            arin_sb = arp.tile([128, ARW], F32, name=f"arin_sb{suffix}")
            for b in range(B):
                xb = inp_b[b]
                if qp_fm is not None:
                    pq = pqk.tile([128, TB], F32, tag="pqk", name=f"pq{suffix}")
                    for kc in range(DC):
                        _mm(nc, pq[:], wqe[:, kc * HM:(kc + 1) * HM],
                            xb[:, kc * TB:(kc + 1) * TB], kc == 0, kc == DC - 1)
                    nc.vector.tensor_scalar(qp_fm[:, b * TB:(b + 1) * TB], pq[:],
                                            0.0, STAB, AluOpType.max, AluOpType.add)
                pk = pqk.tile([128, TB], F32, tag="pqk", name=f"pk{suffix}")
                for kc in range(DC):
                    _mm(nc, pk[:], wke[:, kc * HM:(kc + 1) * HM],
                        xb[:, kc * TB:(kc + 1) * TB], kc == 0, kc == DC - 1)
                kpf = work.tile([128, TB], F32R, tag="kpf", name=f"kpf{suffix}")
                nc.vector.tensor_scalar(kpf[:], pk[:], 0.0, STAB,
                                        AluOpType.max, AluOpType.add)
                # global k-feature sum for z (per-batch column of arin)
                nc.vector.tensor_reduce(arin_sb[:, DC * 512 + b:DC * 512 + b + 1],
                                        kpf[:], AX.X, AluOpType.add)
                # kp token-major [128 tok, HM] per 128-token block (bf16: the
                # ST statistic matmuls run 1 cyc/row at 128-col ap in bf16)
                kptm = kptp.tile([128, NTS * 128], BF16, tag="kptm",
                                 name=f"kptm{suffix}")
                for ts in range(NTS):
                    ptt = tpp.tile([128, 128], F32R, tag="tp", name=f"ptt{suffix}")
                    nc.tensor.transpose(ptt[:], kpf[:, ts * 128:(ts + 1) * 128],
                                        ident[:])
                    nc.any.tensor_copy(kptm[:, ts * 128:(ts + 1) * 128], ptt[:])
                # ST[d, hm] += x[l, d]*kp[l, hm]: transpose x chunks, accumulate
                stA = stps.tile([128, 512], F32, tag="st", name=f"stA{suffix}")
                stB = stps.tile([128, 512], F32, tag="st", name=f"stB{suffix}")
                for kc in range(DC):
                    stt = stA if kc < 4 else stB
                    col = (kc % 4) * 128
                    for ts in range(NTS):
                        xtt = tpp.tile([128, 128], F32R, tag="tp",
                                       name=f"xtt{suffix}")
                        nc.tensor.transpose(
                            xtt[:], xb[:, kc * TB + ts * 128:kc * TB + ts * 128 + 128],
                            ident[:])
                        xts = work.tile([128, 128], BF16, tag="xts",
                                        name=f"xts{suffix}")
                        nc.any.tensor_copy(xts[:], xtt[:])
                        _mmb(nc, stt[:, col:col + 128], xts[:],
                             kptm[:, ts * 128:(ts + 1) * 128],
                             ts == 0, ts == NTS - 1, True)
                for kc in range(DC):
                    stt = stA if kc < 4 else stB
                    col = (kc % 4) * 128
                    nc.any.tensor_copy(
                        arin_sb[:, kc * 512 + b * 128:kc * 512 + b * 128 + 128],
                        stt[:, col:col + 128])
            nc.sync.dma_start(arin[:], arin_sb[:])

    def attn_out_phase(wv_d, wo_d, arout, qp_fm, inp_b, out_pool, out_tag,
                       out_dt, ln_idx, suffix):
        """kv reconstruction + o = qps @ kvW + residual + LN -> out tiles."""
        out_b_list = []
        with ExitStack() as ph:
            gps = ph.enter_context(tc.tile_pool(name=f"gps{suffix}", bufs=3, space="PSUM"))
            sps = ph.enter_context(tc.tile_pool(name=f"sps{suffix}", bufs=5, space="PSUM"))
            ksp = ph.enter_context(tc.tile_pool(name=f"ksp{suffix}", bufs=1))
            kvwp = ph.enter_context(tc.tile_pool(name=f"kvwp{suffix}", bufs=1))

            ksums = ksp.tile([128, 4], F32, name=f"ksums{suffix}")
            kvw = kvwp.tile([128, B * D], BF16, name=f"kvw{suffix}")
            with ExitStack() as s1:
                kvbp = s1.enter_context(tc.tile_pool(name=f"kvbp{suffix}", bufs=1))
                kvbd = kvbp.tile([128, DC * 512], BF16, name=f"kvbd{suffix}")
                with ExitStack() as s0:
                    stbp = s0.enter_context(tc.tile_pool(name=f"stbp{suffix}", bufs=1))
                    st_b = stbp.tile([128, DC * 512], BF16, name=f"stb{suffix}")
                    # load ST (f32 staging freed before weights arrive)
                    with ExitStack() as sg:
                        stg = sg.enter_context(tc.tile_pool(name=f"stg{suffix}", bufs=1))
                        st_f = stg.tile([128, ARW], F32, name=f"stf{suffix}")
                        nc.sync.dma_start(st_f[:], arout[:])
                        nc.any.tensor_copy(ksums[:],
                                           st_f[:, DC * 512:DC * 512 + 4])
                        nc.any.tensor_copy(st_b[:], st_f[:, 0:DC * 512])
                    # stage 1: kvT (masked per batch) with Wv resident
                    with ExitStack() as sv:
                        wvp = sv.enter_context(tc.tile_pool(name=f"wvp{suffix}", bufs=1))
                        wv_sb = load_wide(wvp, wv_d, D, f"wv{suffix}", BF16)
                        kvm_sb = load_wide(wvp, tensors['kvm4'], HM,
                                           f"kvm{suffix}", BF16)
                        for kc in range(DC):
                            pkv = gps.tile([128, 512], F32, tag="g",
                                           name=f"pkv{suffix}")
                            for dc in range(DC):
                                _mmb(nc, pkv[:],
                                     wv_sb[:, dc * D + kc * 128:dc * D + kc * 128 + 128],
                                     st_b[:, dc * 512:(dc + 1) * 512],
                                     dc == 0, dc == DC - 1)
                            for b in range(B):
                                nc.vector.tensor_tensor(
                                    kvbd[:, kc * 512 + b * 128:kc * 512 + b * 128 + 128],
                                    pkv[:, b * 128:(b + 1) * 128],
                                    kvm_sb[:, kc * HM:(kc + 1) * HM],
                                    AluOpType.mult)
                # stage 2: kvW = kvbd^T @ Wo per batch, with Wo resident
                with ExitStack() as s2:
                    wop = s2.enter_context(tc.tile_pool(name=f"wop{suffix}", bufs=1))
                    wo_sb = load_wide(wop, wo_d, D, f"wo{suffix}", BF16)
                    for b in range(B):
                        for half in range(2):
                            pw = gps.tile([128, 512], F32, tag="g",
                                          name=f"pw{suffix}")
                            for kc in range(DC):
                                _mmb(nc, pw[:],
                                     kvbd[:, kc * 512 + b * 128:kc * 512 + b * 128 + 128],
                                     wo_sb[:, kc * D + half * 512:kc * D + half * 512 + 512],
                                     kc == 0, kc == DC - 1)
                            nc.any.tensor_copy(
                                kvw[:, b * D + half * 512:b * D + half * 512 + 512],
                                pw[:])

            # stage 3: z, o, residual, LN per batch
            fv = ph.enter_context(tc.tile_pool(name=f"fv{suffix}", bufs=2))
            r1p = ph.enter_context(tc.tile_pool(name=f"r1{suffix}", bufs=1))
            sqp = ph.enter_context(tc.tile_pool(name=f"sq{suffix}", bufs=2))
            stp = ph.enter_context(tc.tile_pool(name=f"stt{suffix}", bufs=1))
            for b in range(B):
                qpk = fv.tile([128, TB], F32R, tag="qpk", name=f"qpk{suffix}")
                nc.vector.tensor_scalar(qpk[:], qp_fm[:, b * TB:(b + 1) * TB],
                                        ksums[:, b:b + 1], None, AluOpType.mult)
                zps = sps.tile([H, TB], F32, tag="s", name=f"z{suffix}")
                _mm(nc, zps[:], e16T[:], qpk[:], True, True)
                rz = fv.tile([H, TB], F32R, tag="rz", name=f"rz{suffix}")
                nc.vector.reciprocal(rz[:], zps[:])
                t1 = fv.tile([H, TB], F32, tag="nt1", name=f"nt1{suffix}")
                nc.vector.tensor_tensor(t1[:], zps[:], rz[:], AluOpType.mult)
                nc.vector.tensor_scalar(t1[:], t1[:], -1.0, 2.0,
                                        AluOpType.mult, AluOpType.add)
                nc.vector.tensor_tensor(rz[:], rz[:], t1[:], AluOpType.mult)
                zbc = sps.tile([128, TB], F32, tag="s", name=f"zbc{suffix}")
                _mm(nc, zbc[:], e16[:], rz[:], True, True)
                qps_t = fv.tile([128, TB], BF16, tag="qps", name=f"qps{suffix}")
                nc.vector.tensor_tensor(qps_t[:], qp_fm[:, b * TB:(b + 1) * TB],
                                        zbc[:], AluOpType.mult)

                r1 = r1p.tile([128, DC * TB], F32R, tag="r1", name=f"r1{suffix}")
                Sp = sps.tile([1, TB], F32, tag="s", name=f"S{suffix}")
                SSp = sps.tile([1, TB], F32, tag="s", name=f"SS{suffix}")
                for mc in range(DC):
                    po = gps.tile([128, TB], F32, tag="g", name=f"po{suffix}")
                    _mmb(nc, po[:], kvw[:, b * D + mc * 128:b * D + mc * 128 + 128],
                         qps_t[:], True, True)
                    nc.vector.tensor_tensor(r1[:, mc * TB:(mc + 1) * TB], po[:],
                                            inp_b[b][:, mc * TB:(mc + 1) * TB],
                                            AluOpType.add)
                    sq = sqp.tile([128, TB], F32R, tag="sq", name=f"sq{suffix}")
                    nc.scalar.activation(sq[:], r1[:, mc * TB:(mc + 1) * TB],
                                         AF.Square)
                    _mm(nc, Sp[:], ones_col[:, 0:1], r1[:, mc * TB:(mc + 1) * TB],
                        mc == 0, mc == DC - 1, True)
                    _mm(nc, SSp[:], ones_col[:, 0:1], sq[:], mc == 0, mc == DC - 1,
                        True)

                mneg = stp.tile([1, TB], F32, tag="s0", name=f"mneg{suffix}")[:]
                # m2 is dead once ve is computed; n1 reuses its buffer (same
                # tag, bufs=1) to stay inside the SBUF budget
                m2 = stp.tile([1, TB], F32, tag="s1", name=f"m2{suffix}")[:]
                ve = stp.tile([1, TB], F32, tag="s2", name=f"ve{suffix}")[:]
                sqv = stp.tile([1, TB], F32, tag="s3", name=f"sqv{suffix}")[:]
                n1 = stp.tile([1, TB], F32, tag="s1", name=f"n1{suffix}")[:]
                n2 = stp.tile([1, TB], F32, tag="s5", name=f"n2{suffix}")[:]
                a_ = stp.tile([1, TB], F32R, tag="sta", name=f"a{suffix}")
                bb = stp.tile([1, TB], F32R, tag="stb", name=f"bb{suffix}")
                nc.vector.tensor_scalar(mneg, Sp[:], -1.0 / D, None,
                                        AluOpType.mult)
                nc.vector.tensor_tensor(m2, mneg, mneg, AluOpType.mult)
                nc.vector.scalar_tensor_tensor(ve, in0=SSp[:], scalar=1.0 / D,
                                               in1=m2, op0=AluOpType.mult,
                                               op1=AluOpType.subtract)
                nc.scalar.activation(sqv, ve, AF.Sqrt, bias=eps_t[:])
                nc.vector.reciprocal(a_, sqv)
                nc.vector.tensor_tensor(n1, a_, a_, AluOpType.mult)
                nc.vector.scalar_tensor_tensor(n2, in0=ve, scalar=EPS_LN,
                                               in1=n1, op0=AluOpType.add,
                                               op1=AluOpType.mult)
                nc.vector.tensor_scalar(n2, n2, -0.5, 1.5,
                                        AluOpType.mult, AluOpType.add)
                nc.vector.tensor_tensor(a_, a_, n2, AluOpType.mult)
                nc.vector.tensor_tensor(bb, mneg, a_, AluOpType.mult)
                abc = sps.tile([128, TB], F32, tag="s", name=f"abc{suffix}")
                _mm(nc, abc[:], ones_row[:], a_[:], True, True)
                bbc = sps.tile([128, TB], F32, tag="s", name=f"bbc{suffix}")
                _mm(nc, bbc[:], ones_row[:], bb[:], True, True)

                ob = out_pool.tile([128, DC * TB], out_dt, tag=out_tag,
                                   name=f"o{out_tag}{suffix}{b}")
                out_b_list.append(ob)
                for mc in range(DC):
                    tpm = sqp.tile([128, TB], F32, tag="sq", name=f"tpm{suffix}")
                    nc.vector.tensor_tensor(tpm[:], r1[:, mc * TB:(mc + 1) * TB],
                                            abc[:], AluOpType.mult)
                    nc.vector.tensor_tensor(tpm[:], tpm[:], bbc[:], AluOpType.add)
                    nc.scalar.activation(ob[:, mc * TB:(mc + 1) * TB], tpm[:],
                                         AF.Identity,
                                         bias=beslice(ln_idx)[:, mc:mc + 1],
                                         scale=gslice(ln_idx)[:, mc:mc + 1])
        return out_b_list

    def allreduce(arin, arout):
        nc.gpsimd.collective_compute(
            "AllReduce", AluOpType.add,
            replica_groups=[list(range(NCORES))],
            ins=[arin[:]], outs=[arout[:]])

    # ================= attention 1 + 2 =================
    # o2 pool opens first so it can outlive the mid pools (LIFO release)
    o2p = ExitStack()
    o2pool = o2p.enter_context(tc.tile_pool(name=f"o2p{sfx}", bufs=4))
    mid = ExitStack()
    resid = mid.enter_context(tc.tile_pool(name=f"resid{sfx}", bufs=5))
    qpp = mid.enter_context(tc.tile_pool(name=f"qpp{sfx}", bufs=2))

    wqk1 = ExitStack()
    wqkp = wqk1.enter_context(tc.tile_pool(name=f"wqk{sfx}", bufs=1))
    wqe1 = load_wide(wqkp, tensors['wqe1'], HM, f"wqe1{sfx}")
    wke1 = load_wide(wqkp, tensors['wke1'], HM, f"wke1{sfx}")
    wqe2 = load_wide(wqkp, tensors['wqe2'], HM, f"wqe2{sfx}")

    x_b = []
    for b in range(B):
        xb = resid.tile([128, DC * TB], F32R, tag="resid", name=f"x{sfx}{b}")
        for kc in range(DC):
            nc.sync.dma_start(xb[:, kc * TB:(kc + 1) * TB],
                              tensors['xT'][kc * 128:(kc + 1) * 128,
                                            b * TB:(b + 1) * TB])
        x_b.append(xb)

    qp1 = qpp.tile([HM, T], F32R, tag="qp", name=f"qp1{sfx}")
    feat_phase(x_b, wqe1, wke1, qp1, arin1, f"a1{sfx}")
    allreduce(arin1, arout1)

    # overlap with AR1: qp2 from enc_output (streamed)
    qp2 = qpp.tile([HM, T], F32R, tag="qp", name=f"qp2{sfx}")
    with ExitStack() as ph:
        ep = ph.enter_context(tc.tile_pool(name=f"encp{sfx}", bufs=2))
        pqs = ph.enter_context(tc.tile_pool(name=f"pq2{sfx}", bufs=2, space="PSUM"))
        for b in range(B):
            eb = ep.tile([128, DC * TB], F32R, tag="enc", name=f"enc{sfx}")
            for kc in range(DC):
                nc.sync.dma_start(eb[:, kc * TB:(kc + 1) * TB],
                                  tensors['encT'][kc * 128:(kc + 1) * 128,
                                                  b * TB:(b + 1) * TB])
            pq = pqs.tile([128, TB], F32, tag="pq2", name=f"pq2{sfx}")
            for kc in range(DC):
                _mm(nc, pq[:], wqe2[:, kc * HM:(kc + 1) * HM],
                    eb[:, kc * TB:(kc + 1) * TB], kc == 0, kc == DC - 1)
            nc.vector.tensor_scalar(qp2[:, b * TB:(b + 1) * TB], pq[:],
                                    0.0, STAB, AluOpType.max, AluOpType.add)
    wqk1.close()

    out1_b = attn_out_phase(tensors['wv1'], tensors['wo1'], arout1, qp1, x_b,
                            resid, "resid", F32R, 0, f"a1{sfx}")

    wqk2 = ExitStack()
    wqkp2 = wqk2.enter_context(tc.tile_pool(name=f"wk2p{sfx}", bufs=1))
    wke2 = load_wide(wqkp2, tensors['wke2'], HM, f"wke2{sfx}")
    feat_phase(out1_b, None, wke2, None, arin2, f"a2{sfx}")
    allreduce(arin2, arout2)
    wqk2.close()

    out2_b = attn_out_phase(tensors['wv2'], tensors['wo2'], arout2, qp2,
                            out1_b, o2pool, "o2", BF16, 1, f"a2{sfx}")
    mid.close()

    # ================= FFN (h = elu(out2 @ W1 + b1), spilled bf16) ==========
    SL = 512
    NSL = T // SL
    # W2 pool opens (and its DMA issues) before FFN1 so the 8MB load overlaps
    # the h-stage compute instead of serializing the FFN1->FFN2 transition.
    w2scope = ExitStack()
    w2pool = w2scope.enter_context(tc.tile_pool(name=f"w2p{sfx}", bufs=1))
    w2_sb = load_wide(w2pool, tensors['w2'], D, f"w2{sfx}", BF16)
    with ExitStack() as ph:
        wp = ph.enter_context(tc.tile_pool(name=f"w1p{sfx}", bufs=1))
        hstg = ph.enter_context(tc.tile_pool(name=f"hstg{sfx}", bufs=3))
        ep_ = ph.enter_context(tc.tile_pool(name=f"ep{sfx}", bufs=3))
        hps = ph.enter_context(tc.tile_pool(name=f"hps{sfx}", bufs=6, space="PSUM"))
        w1_sb = load_wide(wp, tensors['w1'], DFF, f"w1{sfx}", BF16)
        for s in range(NSL):
            b = s // (TB // SL)
            off = (s % (TB // SL)) * SL
            o2b = out2_b[b]
            for dffc in range(DFC):
                hps_t = hps.tile([128, SL], F32, tag="h", name=f"hps{sfx}")
                for kc in range(DC):
                    _mmb(nc, hps_t[:],
                         w1_sb[:, kc * DFF + dffc * 128:kc * DFF + dffc * 128 + 128],
                         o2b[:, kc * TB + off:kc * TB + off + SL],
                         kc == 0, kc == DC - 1)
                # ELU(u + b1) = min(exp(u+b1) - 1, max(u+b1, 0))
                e_ = ep_.tile([128, SL], F32, tag="e", name=f"e{sfx}")
                nc.scalar.activation(e_[:], hps_t[:], AF.Exp,
                                     bias=b1c[:, dffc:dffc + 1])
                t_ = ep_.tile([128, SL], F32, tag="t", name=f"t{sfx}")
                nc.scalar.activation(t_[:], hps_t[:], AF.Relu,
                                     bias=b1c[:, dffc:dffc + 1])
                h_ = hstg.tile([128, SL], BF16, tag="hsb", name=f"h{sfx}")
                nc.vector.scalar_tensor_tensor(h_[:], in0=e_[:], scalar=1.0,
                                               in1=t_[:], op0=AluOpType.subtract,
                                               op1=AluOpType.min)
                nc.sync.dma_start(
                    h_spill[dffc * 128:(dffc + 1) * 128,
                            s * SL:(s + 1) * SL], h_[:])

    # ============ r3 = h @ W2 + b2 + out2 ; token-major LN3 -> out ==========
    with ExitStack() as ph:
        hin = ph.enter_context(tc.tile_pool(name=f"hin{sfx}", bufs=2))
        r3p = ph.enter_context(tc.tile_pool(name=f"r3p{sfx}", bufs=2))
        o3p = ph.enter_context(tc.tile_pool(name=f"o3p{sfx}", bufs=2))
        sqp = ph.enter_context(tc.tile_pool(name=f"sq3{sfx}", bufs=1))
        stp = ph.enter_context(tc.tile_pool(name=f"st3{sfx}", bufs=8))
        rps = ph.enter_context(tc.tile_pool(name=f"rps{sfx}", bufs=4, space="PSUM"))
        ops = ph.enter_context(tc.tile_pool(name=f"ops{sfx}", bufs=2, space="PSUM"))

        for s in range(NSL):
            b = s // (TB // SL)
            off = (s % (TB // SL)) * SL
            o2b = out2_b[b]
            h_sb = hin.tile([128, DFC * SL], BF16, tag="hin", name=f"hin{sfx}")
            for dffc in range(DFC):
                nc.sync.dma_start(h_sb[:, dffc * SL:(dffc + 1) * SL],
                                  h_spill[dffc * 128:(dffc + 1) * 128,
                                          s * SL:(s + 1) * SL])
            for t3 in range(SL // 128):
                toff = off + t3 * 128
                tok0 = b * TB + toff
                rt = [rps.tile([128, 512], F32, tag="r3", name=f"r3{sfx}{half}")
                      for half in range(2)]
                for half in range(2):
                    for dffc in range(DFC):
                        _mmb(nc, rt[half][:],
                             h_sb[:, dffc * SL + t3 * 128:dffc * SL + t3 * 128 + 128],
                             w2_sb[:, dffc * D + half * 512:dffc * D + half * 512 + 512],
                             dffc == 0, False, True)
                    _mm(nc, rt[half][:], ones_row[:],
                        b2r[:, half * 512:half * 512 + 512], False, True, True)
                # transpose out2 block (bf16) for the token-major residual
                o2t = ops.tile([128, D], BF16, tag="o2t", name=f"o2t{sfx}")
                for kc in range(DC):
                    nc.tensor.matmul(o2t[:, kc * 128:(kc + 1) * 128],
                                     o2b[:, kc * TB + toff:kc * TB + toff + 128],
                                     identb[:], start=True, stop=True,
                                     is_transpose=True, skip_group_check=True)
                o2ts = o3p.tile([128, D], BF16, tag="o2ts", name=f"o2ts{sfx}")
                nc.any.tensor_copy(o2ts[:], o2t[:])
                r3 = r3p.tile([128, D], F32, tag="r3s", name=f"r3s{sfx}")
                for half in range(2):
                    nc.vector.tensor_tensor(r3[:, half * 512:(half + 1) * 512],
                                            rt[half][:],
                                            o2ts[:, half * 512:(half + 1) * 512],
                                            AluOpType.add)
                # token-major LN3: stats along the free (feature) dim
                Sc = stp.tile([128, 1], F32, tag="st3", name=f"Sc{sfx}")
                nc.vector.tensor_reduce(Sc[:], r3[:], AX.X, AluOpType.add)
                mneg = stp.tile([128, 1], F32, tag="st3", name=f"mneg3{sfx}")
                nc.vector.tensor_scalar(mneg[:], Sc[:], -1.0 / D, None,
                                        AluOpType.mult)
                sq = sqp.tile([128, D], F32R, tag="sq3", name=f"sq3{sfx}")
                nc.scalar.activation(sq[:], r3[:], AF.Square)
                SSc = stp.tile([128, 1], F32, tag="st3", name=f"SSc{sfx}")
                nc.vector.tensor_reduce(SSc[:], sq[:], AX.X, AluOpType.add)
                m2 = stp.tile([128, 1], F32, tag="st3", name=f"m23{sfx}")
                nc.vector.tensor_tensor(m2[:], mneg[:], mneg[:], AluOpType.mult)
                ve = stp.tile([128, 1], F32, tag="st3", name=f"ve3{sfx}")
                nc.vector.scalar_tensor_tensor(ve[:], in0=SSc[:], scalar=1.0 / D,
                                               in1=m2[:], op0=AluOpType.mult,
                                               op1=AluOpType.subtract)
                sqv = stp.tile([128, 1], F32, tag="st3", name=f"sqv3{sfx}")
                nc.scalar.activation(sqv[:], ve[:], AF.Sqrt, bias=eps_c[:])
                a_ = stp.tile([128, 1], F32, tag="st3", name=f"a3{sfx}")
                nc.vector.reciprocal(a_[:], sqv[:])
                n1 = stp.tile([128, 1], F32, tag="st3", name=f"n13{sfx}")
                nc.vector.tensor_tensor(n1[:], a_[:], a_[:], AluOpType.mult)
                n2 = stp.tile([128, 1], F32, tag="st3", name=f"n23{sfx}")
                nc.vector.scalar_tensor_tensor(n2[:], in0=ve[:], scalar=EPS_LN,
                                               in1=n1[:], op0=AluOpType.add,
                                               op1=AluOpType.mult)
                nc.vector.tensor_scalar(n2[:], n2[:], -0.5, 1.5,
                                        AluOpType.mult, AluOpType.add)
                nc.vector.tensor_tensor(a_[:], a_[:], n2[:], AluOpType.mult)
                # apply: out = ((r3 - m) * rstd) * g3 + be3
                o3 = o3p.tile([128, D], F32, tag="o3", name=f"o3{sfx}")
                nc.vector.tensor_scalar(o3[:], r3[:], mneg[:], a_[:],
                                        AluOpType.add, AluOpType.mult)
                nc.gpsimd.tensor_tensor(o3[:], o3[:], g3bc[:], AluOpType.mult)
                nc.gpsimd.tensor_tensor(o3[:], o3[:], be3bc[:], AluOpType.add)
                nc.sync.dma_start(out_d[tok0:tok0 + 128, :], o3[:])
    w2scope.close()
    o2p.close()


def _host_prep(inputs):
    """Build per-core in_maps from full inputs."""
    f32 = np.float32
    bf16 = ml_dtypes.bfloat16
    x = np.asarray(inputs['x'], f32)
    enc = np.asarray(inputs['enc_output'], f32)

    def fold(Wq, P):
        # [D, H, DH] x [M, DH] -> [D, H*M]
        w = np.einsum('dhk,mk->dhm', np.asarray(Wq, f32), np.asarray(P, f32))
        return np.ascontiguousarray(w.reshape(D, HM) / np.sqrt(M))

    e16T = np.zeros((HM, H), f32)
    e16 = np.zeros((H, HM), f32)
    for h in range(H):
        e16T[h * M:(h + 1) * M, h] = 1.0
        e16[h, h * M:(h + 1) * M] = 1.0

    khead = np.arange(D) // DH         # head of each v/k feature row
    fhead = np.arange(HM) // M         # head of each random feature
    kvm4 = (khead[:, None] == fhead[None, :]).astype(bf16)

    gbe = np.zeros((128, 4 * DC), f32)
    for i, nm in enumerate(['g1', 'be1', 'g2', 'be2']):
        gbe[:, i * DC:(i + 1) * DC] = np.asarray(inputs[nm], f32).reshape(DC, 128).T

    shared = {
        'wqe1': fold(inputs['Wq1'], inputs['P1']),
        'wke1': fold(inputs['Wk1'], inputs['P1']),
        'wqe2': fold(inputs['Wq2'], inputs['P2']),
        'wke2': fold(inputs['Wk2'], inputs['P2']),
        'wv1': np.asarray(inputs['Wv1'], f32).reshape(D, D).astype(bf16),
        'wo1': np.asarray(inputs['Wo1'], f32).reshape(D, D).astype(bf16),
        'wv2': np.asarray(inputs['Wv2'], f32).reshape(D, D).astype(bf16),
        'wo2': np.asarray(inputs['Wo2'], f32).reshape(D, D).astype(bf16),
        'w1': np.asarray(inputs['W1'], f32).astype(bf16),
        'w2': np.asarray(inputs['W2'], f32).astype(bf16),
        'kvm4': kvm4,
        'e16T': e16T, 'e16': e16,
        'gbe': gbe,
        'b1c': np.ascontiguousarray(np.asarray(inputs['b1'], f32).reshape(DFC, 128).T),
        'b2r': np.asarray(inputs['b2'], f32).reshape(1, D),
        'g3r': np.asarray(inputs['g3'], f32).reshape(1, D),
        'be3r': np.asarray(inputs['be3'], f32).reshape(1, D),
        'ident': np.eye(128, dtype=f32),
        'identb': np.eye(128, dtype=bf16),
        'ones_col': np.ones((128, 8), f32),
        'ones_row': np.ones((1, 128), f32),
    }

    in_maps = []
    for i in range(NCORES):
        sl = slice(i * LSH, (i + 1) * LSH)
        m = dict(shared)
        m['xT'] = np.ascontiguousarray(
            x[:, sl, :].transpose(2, 0, 1).reshape(D, T))
        m['encT'] = np.ascontiguousarray(
            enc[:, sl, :].transpose(2, 0, 1).reshape(D, T))
        in_maps.append(m)
    return in_maps


def kernel(**inputs) -> np.ndarray:
    if 'nc' not in _cache:
        _cache['nc'] = build_program()
    nc = _cache['nc']
    in_maps = _host_prep(inputs)
    res = run_bass_kernel_spmd(nc, in_maps, core_ids=list(range(NCORES)))
    out = np.empty((B, L, D), np.float32)
    for i in range(NCORES):
        o = res.results[i]['out']  # [T, D] token-major
        out[:, i * LSH:(i + 1) * LSH, :] = o.reshape(B, LSH, D)
    return out


if __name__ == '__main__':
    print("building program...")
    build_program()
    print("OK")



# revision 10
# speedup vs baseline: 1.1184x; 1.1184x over previous
"""Trainium2 Bass kernel for nn_DecoderLayer (Performer/FAVOR+ decoder layer).

v3: PE-cycle-lean version of the folded-projection FAVOR+ kernel.

Key ideas on top of v2:
- All matmuls run bf16 (f32r streams ~1.45 cyc/row vs bf16's 1.0).
- kv/kvW reconstruction exploits the per-head block-diagonal mask: instead of
  the full DxD product (masked afterwards, wasting 15/16 of the work), each
  128-row d-chunk kc holds exactly heads (2kc, 2kc+1), so kv is computed only
  for those heads' feature columns (N=64 per chunk) and kvW needs one
  [128,64]x[128,512] matmul per (chunk, half) with a zero-padded lhsT.
- LN1/LN2 token-sum S comes from one matmul with the kvw row-sum vector
  (sum_d po = (sum_d kvw)^T qps) plus a host-computed residual feature-sum
  (attn1) or the be1-sum constant (attn2), replacing 8 accumulated matmuls.
- FFN biases fold into existing elementwise ops (b2 via a broadcast tile,
  b1 via the ELU's activation bias) - no ones-row bias matmuls.
- attn1's ST statistic uses a host-provided token-major x copy instead of
  32 PE transposes per batch.
- Elementwise work in the attention epilogue is spread across DVE /
  Activation / GpSimd so the tensor engine stays busy (p-state residency).

Sharding: sequence (L) split across 8 cores; the per-batch global statistics
ST = [X^T kp | kp-sums] (~2MB total for B=4) are AllReduced twice.
Residual stream is feature-major; the final LN3 runs token-major so the
output DMAs out in natural [T, D] layout (no host post-transpose).
"""
import sys
import os

sys.path.insert(0, '/opt/trn_rl_repo')

import numpy as np
import ml_dtypes
from contextlib import ExitStack

from concourse import bass, bacc, tile
import concourse.mybir as mybir
from concourse.bass_utils import run_bass_kernel_spmd
from concourse.alu_op_type import AluOpType

F32 = mybir.dt.float32
F32R = mybir.dt.float32r
BF16 = mybir.dt.bfloat16
AF = mybir.ActivationFunctionType
AX = mybir.AxisListType

B, L, D, H, DH, M, DFF = 4, 4096, 1024, 16, 64, 8, 4096
NCORES = 8
LSH = L // NCORES          # 512 tokens of L per core
T = B * LSH                # 2048 tokens per core
TB = LSH                   # tokens per batch element per core (512)
NTS = TB // 128            # 4 x 128-token blocks per batch
DC = D // 128              # 8 d-chunks
DFC = DFF // 128           # 32 dff-chunks
HM = H * M                 # 128 random features total
HPC = 2                    # heads per 128-row d-chunk (128 // DH)
EPS_LN = 1e-6
STAB = 0.001
ARW = DC * 512 + 4         # allreduce width: ST (8 chunks x 4 batches x 128) + ksums

_cache = {}


def _mm(nc, out, lhsT, rhs, start, stop, skip=False):
    nc.tensor.matmul(out, lhsT.bitcast(F32R), rhs.bitcast(F32R),
                     start=start, stop=stop, skip_group_check=skip)


def _mmb(nc, out, lhsT, rhs, start, stop, skip=False):
    nc.tensor.matmul(out, lhsT, rhs, start=start, stop=stop,
                     skip_group_check=skip)


def build_program(loop=1):
    nc = bacc.Bacc("TRN2", target_bir_lowering=False, debug=False,
                   num_devices=NCORES)

    def din(name, shape, dt=F32R):
        return nc.dram_tensor(name, shape, dt, kind="ExternalInput").ap()

    tensors = dict(
        xT=din("xT", [D, T], BF16),
        encT=din("encT", [D, T], BF16),
        xtm=din("xtm", [T, D], BF16),
        wqe1=din("wqe1", [D, HM], BF16), wke1=din("wke1", [D, HM], BF16),
        wqe2=din("wqe2", [D, HM], BF16), wke2=din("wke2", [D, HM], BF16),
        wv1=din("wv1", [D, D], BF16), wo1=din("wo1", [D, D], BF16),
        wv2=din("wv2", [D, D], BF16), wo2=din("wo2", [D, D], BF16),
        w1=din("w1", [D, DFF], BF16), w2=din("w2", [DFF, D], BF16),
        e16T_d=din("e16T", [HM, H], BF16), e16_d=din("e16", [H, HM], BF16),
        gbe_d=din("gbe", [128, 4 * DC], F32),
        b1c_d=din("b1c", [128, DFC], F32),
        b2r_d=din("b2r", [1, D]),
        g3r_d=din("g3r", [1, D]), be3r_d=din("be3r", [1, D]),
        identb_d=din("identb", [128, 128], BF16),
        ones_col_d=din("ones_col", [128, 8], BF16),
        ones_row_d=din("ones_row", [1, 128]),
        sx_d=din("sx", [1, T], F32),
        sbe1_d=din("sbe1", [1, 1], F32),
    )

    out_d = nc.dram_tensor("out", [T, D], F32, kind="ExternalOutput").ap()

    with nc.allow_low_precision(reason="bf16 matmul inputs"), \
         tile.TileContext(nc) as tc, ExitStack() as top:
        dram = top.enter_context(tc.tile_pool(name="dram", bufs=1, space="DRAM"))

        const = top.enter_context(tc.tile_pool(name="const", bufs=1))
        c = {}
        c['e16T'] = const.tile([HM, H], BF16, name="e16T")
        nc.sync.dma_start(c['e16T'][:], tensors['e16T_d'][:])
        c['e16'] = const.tile([H, HM], BF16, name="e16")
        nc.sync.dma_start(c['e16'][:], tensors['e16_d'][:])
        c['gbe'] = const.tile([128, 4 * DC], F32, name="gbe")
        nc.sync.dma_start(c['gbe'][:], tensors['gbe_d'][:])
        c['b1c'] = const.tile([128, DFC], F32, name="b1c")
        nc.sync.dma_start(c['b1c'][:], tensors['b1c_d'][:])
        c['identb'] = const.tile([128, 128], BF16, name="identb")
        nc.sync.dma_start(c['identb'][:], tensors['identb_d'][:])
        c['ones_col'] = const.tile([128, 8], BF16, name="ones_col")
        nc.sync.dma_start(c['ones_col'][:], tensors['ones_col_d'][:])
        c['ones_row'] = const.tile([1, 128], F32R, name="ones_row")
        nc.sync.dma_start(c['ones_row'][:], tensors['ones_row_d'][:])
        c['sbe1'] = const.tile([1, 1], F32, name="sbe1")
        nc.sync.dma_start(c['sbe1'][:], tensors['sbe1_d'][:])
        c['eps_t'] = const.tile([1, 1], F32, name="eps_t")
        nc.vector.memset(c['eps_t'][:], EPS_LN)
        c['eps_c'] = const.tile([128, 1], F32, name="eps_c")
        nc.vector.memset(c['eps_c'][:], EPS_LN)
        c['g3bc'] = const.tile([128, D], F32R, name="g3bc")
        c['be3bc'] = const.tile([128, D], F32R, name="be3bc")
        c['b2bc'] = const.tile([128, D], F32R, name="b2bc")
        # row-constant sources live in a transient pool so their [1, D]
        # allocations don't hold per-partition SBUF for the whole kernel
        with tc.tile_pool(name="rowc", bufs=1) as rowp, \
             tc.tile_pool(name="bc_ps", bufs=2, space="PSUM") as bcp:
            for srcd, dstk in ((tensors['b2r_d'], 'b2bc'),
                               (tensors['g3r_d'], 'g3bc'),
                               (tensors['be3r_d'], 'be3bc')):
                src = rowp.tile([1, D], F32R, tag="rowc", name=f"r{dstk}")
                nc.sync.dma_start(src[:], srcd[:])
                for half in range(2):
                    pg = bcp.tile([128, D // 2], F32, tag="bc", name="pbc")
                    _mm(nc, pg[:], c['ones_row'][:],
                        src[:, half * 512:(half + 1) * 512], True, True)
                    nc.any.tensor_copy(c[dstk][:, half * 512:(half + 1) * 512],
                                       pg[:])

        for it in range(loop):
            build_iter(nc, tc, f"i{it}" if loop > 1 else "", tensors, c,
                       dram, out_d)

    nc.compile()
    return nc


def build_iter(nc, tc, sfx, tensors, c, dram, out_d):
    e16T, e16 = c['e16T'], c['e16']
    gbe, b1c = c['gbe'], c['b1c']
    identb = c['identb']
    ones_col, ones_row = c['ones_col'], c['ones_row']
    eps_t, g3bc, be3bc, b2bc = c['eps_t'], c['g3bc'], c['be3bc'], c['b2bc']
    eps_c = c['eps_c']

    arin1 = dram.tile([128, ARW], F32, name=f"arin1{sfx}")
    arout1 = dram.tile([128, ARW], F32, addr_space="Shared", name=f"arout1{sfx}")
    arin2 = dram.tile([128, ARW], F32, name=f"arin2{sfx}")
    arout2 = dram.tile([128, ARW], F32, addr_space="Shared", name=f"arout2{sfx}")
    h_spill = dram.tile([DFF, T], BF16, name=f"hspill{sfx}")

    def gslice(i):
        return gbe[:, 2 * i * DC:(2 * i + 1) * DC]

    def beslice(i):
        return gbe[:, (2 * i + 1) * DC:(2 * i + 2) * DC]

    def load_wide(pool, src_dram, ncols, name, dt=BF16):
        nchunk = src_dram.shape[0] // 128
        t_ = pool.tile([128, nchunk * ncols], dt, name=name)
        for kc in range(nchunk):
            nc.sync.dma_start(t_[:, kc * ncols:(kc + 1) * ncols],
                              src_dram[kc * 128:(kc + 1) * 128, :])
        return t_

    def feat_phase(inp_b, wqe, wke, qp_fm, arin, xtm_d, suffix):
        """qp/kp features + ST statistic + ksum -> arin (DRAM).

        xtm_d: DRAM [T, D] token-major input for the ST lhsT (attn1), or None
        to transpose inp_b chunks on the PE (attn2).
        """
        with ExitStack() as ph:
            work = ph.enter_context(tc.tile_pool(name=f"ftw{suffix}", bufs=2))
            kptp = ph.enter_context(tc.tile_pool(name=f"kptp{suffix}", bufs=2))
            arp = ph.enter_context(tc.tile_pool(name=f"arp{suffix}", bufs=1))
            pqk = ph.enter_context(tc.tile_pool(name=f"pqk{suffix}", bufs=2, space="PSUM"))
            tpp = ph.enter_context(tc.tile_pool(name=f"tpp{suffix}", bufs=4, space="PSUM"))
            stps = ph.enter_context(tc.tile_pool(name=f"stps{suffix}", bufs=2, space="PSUM"))
            if xtm_d is not None:
                xtp = ph.enter_context(tc.tile_pool(name=f"xtp{suffix}", bufs=2))

            arin_sb = arp.tile([128, ARW], F32, name=f"arin_sb{suffix}")
            for b in range(B):
                xb = inp_b[b]
                if xtm_d is not None:
                    xtm_b = xtp.tile([128, NTS * D], BF16, tag="xtm",
                                     name=f"xtm{suffix}")
                    for ts in range(NTS):
                        nc.sync.dma_start(
                            xtm_b[:, ts * D:(ts + 1) * D],
                            xtm_d[b * TB + ts * 128:b * TB + ts * 128 + 128, :])
                if qp_fm is not None:
                    pq = pqk.tile([128, TB], F32, tag="pqk", name=f"pq{suffix}")
                    for kc in range(DC):
                        _mmb(nc, pq[:], wqe[:, kc * HM:(kc + 1) * HM],
                             xb[:, kc * TB:(kc + 1) * TB], kc == 0, kc == DC - 1)
                    nc.vector.tensor_scalar(qp_fm[:, b * TB:(b + 1) * TB], pq[:],
                                            0.0, STAB, AluOpType.max, AluOpType.add)
                pk = pqk.tile([128, TB], F32, tag="pqk", name=f"pk{suffix}")
                for kc in range(DC):
                    _mmb(nc, pk[:], wke[:, kc * HM:(kc + 1) * HM],
                         xb[:, kc * TB:(kc + 1) * TB], kc == 0, kc == DC - 1)
                kpf = work.tile([128, TB], BF16, tag="kpf", name=f"kpf{suffix}")
                nc.vector.tensor_scalar(kpf[:], pk[:], 0.0, STAB,
                                        AluOpType.max, AluOpType.add)
                # global k-feature sum for z (per-batch column of arin)
                nc.vector.tensor_reduce(arin_sb[:, DC * 512 + b:DC * 512 + b + 1],
                                        kpf[:], AX.X, AluOpType.add)
                # kp token-major [128 tok, HM] per 128-token block
                kptm = kptp.tile([128, NTS * 128], BF16, tag="kptm",
                                 name=f"kptm{suffix}")
                for ts in range(NTS):
                    ptt = tpp.tile([128, 128], BF16, tag="tp", name=f"ptt{suffix}")
                    nc.tensor.transpose(ptt[:], kpf[:, ts * 128:(ts + 1) * 128],
                                        identb[:])
                    nc.any.tensor_copy(kptm[:, ts * 128:(ts + 1) * 128], ptt[:])
                # ST[d, hm] += x[l, d]*kp[l, hm] with token-major lhsT
                stA = stps.tile([128, 512], F32, tag="st", name=f"stA{suffix}")
                stB = stps.tile([128, 512], F32, tag="st", name=f"stB{suffix}")
                for kc in range(DC):
                    stt = stA if kc < 4 else stB
                    col = (kc % 4) * 128
                    for ts in range(NTS):
                        if xtm_d is not None:
                            lhsT = xtm_b[:, ts * D + kc * 128:ts * D + kc * 128 + 128]
                        else:
                            xtt = tpp.tile([128, 128], BF16, tag="tp",
                                           name=f"xtt{suffix}")
                            nc.tensor.transpose(
                                xtt[:], xb[:, kc * TB + ts * 128:kc * TB + ts * 128 + 128],
                                identb[:])
                            lhsT = work.tile([128, 128], BF16, tag="xts",
                                             name=f"xts{suffix}")
                            nc.any.tensor_copy(lhsT[:], xtt[:])
                            lhsT = lhsT[:]
                        _mmb(nc, stt[:, col:col + 128], lhsT,
                             kptm[:, ts * 128:(ts + 1) * 128],
                             ts == 0, ts == NTS - 1, True)
                for kc in range(DC):
                    stt = stA if kc < 4 else stB
                    col = (kc % 4) * 128
                    nc.any.tensor_copy(
                        arin_sb[:, kc * 512 + b * 128:kc * 512 + b * 128 + 128],
                        stt[:, col:col + 128])
            nc.sync.dma_start(arin[:], arin_sb[:])

    def attn_out_phase(wv_d, wo_d, arout, qp_fm, inp_b, out_pool, out_tag,
                       out_dt, ln_idx, suffix, s_resid=None):
        """Block-diagonal kv/kvW + o = qps @ kvW + residual + LN -> out tiles.

        s_resid: [1, T] AP of per-token residual feature-sums (attn1), or None
        to use the be1-sum constant (attn2).
        """
        out_b_list = []
        with ExitStack() as ph:
            gps = ph.enter_context(tc.tile_pool(name=f"gps{suffix}", bufs=3, space="PSUM"))
            sps = ph.enter_context(tc.tile_pool(name=f"sps{suffix}", bufs=5, space="PSUM"))
            ksp = ph.enter_context(tc.tile_pool(name=f"ksp{suffix}", bufs=1))
            kvwp = ph.enter_context(tc.tile_pool(name=f"kvwp{suffix}", bufs=1))

            ksums = ksp.tile([128, 4], F32, name=f"ksums{suffix}")
            kvw = kvwp.tile([128, B * D], BF16, name=f"kvw{suffix}")
            # zero-padded stage-2 lhsT: [128, kc, (b, j)] with cross-head
            # blocks zero (row p of chunk kc belongs to head 2kc + p//64;
            # col j<8 is head 2kc, j>=8 head 2kc+1)
            kvz = ksp.tile([128, DC, B, 2 * M], BF16, name=f"kvz{suffix}")
            nc.vector.memset(kvz[:], 0.0)
            with ExitStack() as s1:
                stbp = s1.enter_context(tc.tile_pool(name=f"stbp{suffix}", bufs=1))
                st_b = stbp.tile([128, DC * 512], BF16, name=f"stb{suffix}")
                with ExitStack() as sg:
                    stg = sg.enter_context(tc.tile_pool(name=f"stg{suffix}", bufs=1))
                    st_f = stg.tile([128, ARW], F32, name=f"stf{suffix}")
                    nc.sync.dma_start(st_f[:], arout[:])
                    nc.any.tensor_copy(ksums[:],
                                       st_f[:, DC * 512:DC * 512 + 4])
                    nc.any.tensor_copy(st_b[:], st_f[:, 0:DC * 512])
                st4 = st_b[:].rearrange("p (dc b hm) -> p dc b hm", dc=DC, b=B)
                # stage 1: block-diag kv: per chunk kc only the 2 resident
                # heads' feature columns (N=64)
                with ExitStack() as sv:
                    wvp = sv.enter_context(tc.tile_pool(name=f"wvp{suffix}", bufs=1))
                    wv_sb = load_wide(wvp, wv_d, D, f"wv{suffix}")
                    for kc in range(DC):
                        pkv = gps.tile([128, B * 2 * M], F32, tag="g",
                                       name=f"pkv{suffix}")
                        for dc in range(DC):
                            _mmb(nc, pkv[:],
                                 wv_sb[:, dc * D + kc * 128:dc * D + kc * 128 + 128],
                                 st4[:, dc, :, 2 * M * kc:2 * M * kc + 2 * M],
                                 dc == 0, dc == DC - 1)
                        pkv3 = pkv[:].rearrange("p (b j) -> p b j", b=B)
                        nc.any.tensor_copy(kvz[0:64, kc, :, 0:M], pkv3[0:64, :, 0:M])
                        nc.any.tensor_copy(kvz[64:128, kc, :, M:2 * M],
                                           pkv3[64:128, :, M:2 * M])
            # stage 2: kvW rows for chunk kc's heads come only from chunk kc.
            # Scatter into kvw's hm-partition layout crosses 32-partition
            # alignment (compute engines require base%32==0), so the per-batch
            # 16-row moves go through an aligned SBUF staging copy + DMA.
            with ExitStack() as s2:
                wop = s2.enter_context(tc.tile_pool(name=f"wop{suffix}", bufs=1))
                stgp = s2.enter_context(tc.tile_pool(name=f"wsg{suffix}", bufs=4))
                wo_sb = load_wide(wop, wo_d, D, f"wo{suffix}")
                for kc in range(DC):
                    for half in range(2):
                        pw = gps.tile([B * 2 * M, 512], F32, tag="g",
                                      name=f"pw{suffix}")
                        _mmb(nc, pw[:], kvz[:, kc, :, :],
                             wo_sb[:, kc * D + half * 512:kc * D + half * 512 + 512],
                             True, True)
                        stg = stgp.tile([B * 2 * M, 512], BF16, tag="wsg",
                                        name=f"wsg{suffix}")
                        nc.any.tensor_copy(stg[:], pw[:])
                        for b in range(B):
                            nc.sync.dma_start(
                                kvw[2 * M * kc:2 * M * (kc + 1),
                                    b * D + half * 512:b * D + half * 512 + 512],
                                stg[2 * M * b:2 * M * (b + 1), :])

            # stage 3: z, o, residual, LN per batch
            fv = ph.enter_context(tc.tile_pool(name=f"fv{suffix}", bufs=2))
            r1p = ph.enter_context(tc.tile_pool(name=f"r1{suffix}", bufs=1))
            sqp = ph.enter_context(tc.tile_pool(name=f"sq{suffix}", bufs=2))
            stp = ph.enter_context(tc.tile_pool(name=f"stt{suffix}", bufs=1))
            for b in range(B):
                # z = per-head sums of qp * ksums, via ks-scaled e16
                e16ks = fv.tile([HM, H], BF16, tag="e16ks", name=f"e16ks{suffix}")
                nc.vector.tensor_scalar(e16ks[:], e16T[:], ksums[:, b:b + 1],
                                        None, AluOpType.mult)
                zps = sps.tile([H, TB], F32, tag="s", name=f"z{suffix}")
                _mmb(nc, zps[:], e16ks[:], qp_fm[:, b * TB:(b + 1) * TB],
                     True, True)
                rz = fv.tile([H, TB], F32R, tag="rz", name=f"rz{suffix}")
                nc.vector.reciprocal(rz[:], zps[:])
                t1 = fv.tile([H, TB], F32, tag="nt1", name=f"nt1{suffix}")
                nc.vector.tensor_tensor(t1[:], zps[:], rz[:], AluOpType.mult)
                nc.vector.tensor_scalar(t1[:], t1[:], -1.0, 2.0,
                                        AluOpType.mult, AluOpType.add)
                nc.vector.tensor_tensor(rz[:], rz[:], t1[:], AluOpType.mult)
                rzb = fv.tile([H, TB], BF16, tag="rzb", name=f"rzb{suffix}")
                nc.any.tensor_copy(rzb[:], rz[:])
                zbc = sps.tile([128, TB], F32, tag="s", name=f"zbc{suffix}")
                _mmb(nc, zbc[:], e16[:], rzb[:], True, True)
                qps_t = fv.tile([128, TB], BF16, tag="qps", name=f"qps{suffix}")
                nc.vector.tensor_tensor(qps_t[:], qp_fm[:, b * TB:(b + 1) * TB],
                                        zbc[:], AluOpType.mult)

                # S = sum_d r1 via kvw row-sums + residual feature-sums
                wbf = fv.tile([128, 1], F32, tag="wbf", name=f"wbf{suffix}")
                nc.vector.tensor_reduce(wbf[:], kvw[:, b * D:(b + 1) * D],
                                        AX.X, AluOpType.add)
                wbb = fv.tile([128, 1], BF16, tag="wbb", name=f"wbb{suffix}")
                nc.any.tensor_copy(wbb[:], wbf[:])
                Sp = sps.tile([1, TB], F32, tag="s", name=f"S{suffix}")
                _mmb(nc, Sp[:], wbb[:], qps_t[:], True, True, True)
                mneg = stp.tile([1, TB], F32, tag="s0", name=f"mneg{suffix}")[:]
                if s_resid is not None:
                    Ssum = stp.tile([1, TB], F32, tag="s4", name=f"Ssum{suffix}")
                    nc.vector.tensor_tensor(Ssum[:], Sp[:],
                                            s_resid[:, b * TB:(b + 1) * TB],
                                            AluOpType.add)
                    nc.vector.tensor_scalar(mneg, Ssum[:], -1.0 / D, None,
                                            AluOpType.mult)
                else:
                    nc.vector.tensor_scalar(mneg, Sp[:], c['sbe1'][:, 0:1],
                                            -1.0 / D, AluOpType.add,
                                            AluOpType.mult)

                r1 = r1p.tile([128, DC * TB], F32R, tag="r1", name=f"r1{suffix}")
                SSp = sps.tile([1, TB], F32, tag="s", name=f"SS{suffix}")
                for mc in range(DC):
                    po = gps.tile([128, TB], F32, tag="g", name=f"po{suffix}")
                    _mmb(nc, po[:], kvw[:, b * D + mc * 128:b * D + mc * 128 + 128],
                         qps_t[:], True, True)
                    nc.vector.tensor_tensor(r1[:, mc * TB:(mc + 1) * TB], po[:],
                                            inp_b[b][:, mc * TB:(mc + 1) * TB],
                                            AluOpType.add)
                    sq = sqp.tile([128, TB], BF16, tag="sq", name=f"sq{suffix}")
                    nc.scalar.activation(sq[:], r1[:, mc * TB:(mc + 1) * TB],
                                         AF.Square)
                    _mmb(nc, SSp[:], ones_col[:, 0:1], sq[:], mc == 0,
                         mc == DC - 1, True)

                # m2 is dead once ve is computed; n1 reuses its buffer (same
                # tag, bufs=1) to stay inside the SBUF budget
                m2 = stp.tile([1, TB], F32, tag="s1", name=f"m2{suffix}")[:]
                ve = stp.tile([1, TB], F32, tag="s2", name=f"ve{suffix}")[:]
                sqv = stp.tile([1, TB], F32, tag="s3", name=f"sqv{suffix}")[:]
                n1 = stp.tile([1, TB], F32, tag="s1", name=f"n1{suffix}")[:]
                n2 = stp.tile([1, TB], F32, tag="s5", name=f"n2{suffix}")[:]
                a_ = stp.tile([1, TB], F32R, tag="sta", name=f"a{suffix}")
                bb = stp.tile([1, TB], F32R, tag="stb", name=f"bb{suffix}")
                nc.vector.tensor_tensor(m2, mneg, mneg, AluOpType.mult)
                nc.vector.scalar_tensor_tensor(ve, in0=SSp[:], scalar=1.0 / D,
                                               in1=m2, op0=AluOpType.mult,
                                               op1=AluOpType.subtract)
                nc.scalar.activation(sqv, ve, AF.Sqrt, bias=eps_t[:])
                nc.vector.reciprocal(a_, sqv)
                nc.vector.tensor_tensor(n1, a_, a_, AluOpType.mult)
                nc.vector.scalar_tensor_tensor(n2, in0=ve, scalar=EPS_LN,
                                               in1=n1, op0=AluOpType.add,
                                               op1=AluOpType.mult)
                nc.vector.tensor_scalar(n2, n2, -0.5, 1.5,
                                        AluOpType.mult, AluOpType.add)
                nc.vector.tensor_tensor(a_, a_, n2, AluOpType.mult)
                nc.vector.tensor_tensor(bb, mneg, a_, AluOpType.mult)
                abc = sps.tile([128, TB], F32, tag="s", name=f"abc{suffix}")
                _mm(nc, abc[:], ones_row[:], a_[:], True, True)
                bbc = sps.tile([128, TB], F32, tag="s", name=f"bbc{suffix}")
                _mm(nc, bbc[:], ones_row[:], bb[:], True, True)
                # gpsimd cannot read PSUM; stage bbc in SBUF for its adds
                bbcs = fv.tile([128, TB], F32, tag="bbcs", name=f"bbcs{suffix}")
                nc.vector.tensor_copy(bbcs[:], bbc[:])

                ob = out_pool.tile([128, DC * TB], out_dt, tag=out_tag,
                                   name=f"o{out_tag}{suffix}{b}")
                out_b_list.append(ob)
                for mc in range(DC):
                    tpm = sqp.tile([128, TB], F32, tag="sq", name=f"tpm{suffix}")
                    nc.vector.tensor_tensor(tpm[:], r1[:, mc * TB:(mc + 1) * TB],
                                            abc[:], AluOpType.mult)
                    if mc < 5:
                        nc.gpsimd.tensor_tensor(tpm[:], tpm[:], bbcs[:],
                                                AluOpType.add)
                    else:
                        nc.vector.tensor_tensor(tpm[:], tpm[:], bbc[:],
                                                AluOpType.add)
                    if mc < 6:
                        nc.scalar.activation(ob[:, mc * TB:(mc + 1) * TB], tpm[:],
                                             AF.Identity,
                                             bias=beslice(ln_idx)[:, mc:mc + 1],
                                             scale=gslice(ln_idx)[:, mc:mc + 1])
                    else:
                        nc.vector.tensor_scalar(ob[:, mc * TB:(mc + 1) * TB],
                                                tpm[:],
                                                gslice(ln_idx)[:, mc:mc + 1],
                                                beslice(ln_idx)[:, mc:mc + 1],
                                                AluOpType.mult, AluOpType.add)
        return out_b_list

    def allreduce(arin, arout):
        nc.gpsimd.collective_compute(
            "AllReduce", AluOpType.add,
            replica_groups=[list(range(NCORES))],
            ins=[arin[:]], outs=[arout[:]])

    # ================= attention 1 + 2 =================
    # o2 pool opens first so it can outlive the mid pools (LIFO release)
    o2p = ExitStack()
    o2pool = o2p.enter_context(tc.tile_pool(name=f"o2p{sfx}", bufs=4))
    mid = ExitStack()
    resid = mid.enter_context(tc.tile_pool(name=f"resid{sfx}", bufs=5))
    qpp = mid.enter_context(tc.tile_pool(name=f"qpp{sfx}", bufs=2))
    sxp = mid.enter_context(tc.tile_pool(name=f"sxp{sfx}", bufs=1))
    sx_sb = sxp.tile([1, T], F32, name=f"sx{sfx}")
    nc.sync.dma_start(sx_sb[:], tensors['sx_d'][:])

    wqk1 = ExitStack()
    wqkp = wqk1.enter_context(tc.tile_pool(name=f"wqk{sfx}", bufs=1))
    wqe1 = load_wide(wqkp, tensors['wqe1'], HM, f"wqe1{sfx}")
    wke1 = load_wide(wqkp, tensors['wke1'], HM, f"wke1{sfx}")
    wqe2 = load_wide(wqkp, tensors['wqe2'], HM, f"wqe2{sfx}")

    x_b = []
    for b in range(B):
        xb = resid.tile([128, DC * TB], BF16, tag="resid", name=f"x{sfx}{b}")
        for kc in range(DC):
            nc.sync.dma_start(xb[:, kc * TB:(kc + 1) * TB],
                              tensors['xT'][kc * 128:(kc + 1) * 128,
                                            b * TB:(b + 1) * TB])
        x_b.append(xb)

    qp1 = qpp.tile([HM, T], BF16, tag="qp", name=f"qp1{sfx}")
    feat_phase(x_b, wqe1, wke1, qp1, arin1, tensors['xtm'], f"a1{sfx}")
    allreduce(arin1, arout1)

    # overlap with AR1: qp2 from enc_output (streamed)
    qp2 = qpp.tile([HM, T], BF16, tag="qp", name=f"qp2{sfx}")
    with ExitStack() as ph:
        ep = ph.enter_context(tc.tile_pool(name=f"encp{sfx}", bufs=2))
        pqs = ph.enter_context(tc.tile_pool(name=f"pq2{sfx}", bufs=2, space="PSUM"))
        for b in range(B):
            eb = ep.tile([128, DC * TB], BF16, tag="enc", name=f"enc{sfx}")
            for kc in range(DC):
                nc.sync.dma_start(eb[:, kc * TB:(kc + 1) * TB],
                                  tensors['encT'][kc * 128:(kc + 1) * 128,
                                                  b * TB:(b + 1) * TB])
            pq = pqs.tile([128, TB], F32, tag="pq2", name=f"pq2{sfx}")
            for kc in range(DC):
                _mmb(nc, pq[:], wqe2[:, kc * HM:(kc + 1) * HM],
                     eb[:, kc * TB:(kc + 1) * TB], kc == 0, kc == DC - 1)
            nc.vector.tensor_scalar(qp2[:, b * TB:(b + 1) * TB], pq[:],
                                    0.0, STAB, AluOpType.max, AluOpType.add)
    wqk1.close()

    out1_b = attn_out_phase(tensors['wv1'], tensors['wo1'], arout1, qp1, x_b,
                            resid, "resid", BF16, 0, f"a1{sfx}",
                            s_resid=sx_sb)

    wqk2 = ExitStack()
    wqkp2 = wqk2.enter_context(tc.tile_pool(name=f"wk2p{sfx}", bufs=1))
    wke2 = load_wide(wqkp2, tensors['wke2'], HM, f"wke2{sfx}")
    feat_phase(out1_b, None, wke2, None, arin2, None, f"a2{sfx}")
    allreduce(arin2, arout2)
    wqk2.close()

    out2_b = attn_out_phase(tensors['wv2'], tensors['wo2'], arout2, qp2,
                            out1_b, o2pool, "o2", BF16, 1, f"a2{sfx}")
    mid.close()

    # ================= FFN (h = elu(out2 @ W1 + b1), spilled bf16) ==========
    SL = 512
    NSL = T // SL
    # W2 pool opens (and its DMA issues) before FFN1 so the 8MB load overlaps
    # the h-stage compute instead of serializing the FFN1->FFN2 transition.
    w2scope = ExitStack()
    w2pool = w2scope.enter_context(tc.tile_pool(name=f"w2p{sfx}", bufs=1))
    w2_sb = load_wide(w2pool, tensors['w2'], D, f"w2{sfx}")
    with ExitStack() as ph:
        wp = ph.enter_context(tc.tile_pool(name=f"w1p{sfx}", bufs=1))
        hstg = ph.enter_context(tc.tile_pool(name=f"hstg{sfx}", bufs=3))
        ep_ = ph.enter_context(tc.tile_pool(name=f"ep{sfx}", bufs=3))
        hps = ph.enter_context(tc.tile_pool(name=f"hps{sfx}", bufs=8, space="PSUM"))
        w1_sb = load_wide(wp, tensors['w1'], DFF, f"w1{sfx}")
        for s in range(NSL):
            b = s // (TB // SL)
            off = (s % (TB // SL)) * SL
            o2b = out2_b[b]
            for dffc in range(DFC):
                hps_t = hps.tile([128, SL], F32, tag="h", name=f"hps{sfx}")
                for kc in range(DC):
                    _mmb(nc, hps_t[:],
                         w1_sb[:, kc * DFF + dffc * 128:kc * DFF + dffc * 128 + 128],
                         o2b[:, kc * TB + off:kc * TB + off + SL],
                         kc == 0, kc == DC - 1)
                # ELU(u + b1) = min(exp(u+b1) - 1, max(u+b1, 0))
                e_ = ep_.tile([128, SL], BF16, tag="e", name=f"e{sfx}")
                nc.scalar.activation(e_[:], hps_t[:], AF.Exp,
                                     bias=b1c[:, dffc:dffc + 1])
                t_ = ep_.tile([128, SL], BF16, tag="t", name=f"t{sfx}")
                nc.vector.tensor_scalar(t_[:], hps_t[:], b1c[:, dffc:dffc + 1],
                                        0.0, AluOpType.add, AluOpType.max)
                h_ = hstg.tile([128, SL], BF16, tag="hsb", name=f"h{sfx}")
                nc.vector.scalar_tensor_tensor(h_[:], in0=e_[:], scalar=1.0,
                                               in1=t_[:], op0=AluOpType.subtract,
                                               op1=AluOpType.min)
                nc.sync.dma_start(
                    h_spill[dffc * 128:(dffc + 1) * 128,
                            s * SL:(s + 1) * SL], h_[:])

    # ============ r3 = h @ W2 + b2 + out2 ; token-major LN3 -> out ==========
    with ExitStack() as ph:
        hin = ph.enter_context(tc.tile_pool(name=f"hin{sfx}", bufs=2))
        r3p = ph.enter_context(tc.tile_pool(name=f"r3p{sfx}", bufs=2))
        o3p = ph.enter_context(tc.tile_pool(name=f"o3p{sfx}", bufs=2))
        sqp = ph.enter_context(tc.tile_pool(name=f"sq3{sfx}", bufs=1))
        stp = ph.enter_context(tc.tile_pool(name=f"st3{sfx}", bufs=8))
        rps = ph.enter_context(tc.tile_pool(name=f"rps{sfx}", bufs=4, space="PSUM"))
        ops = ph.enter_context(tc.tile_pool(name=f"ops{sfx}", bufs=2, space="PSUM"))

        for s in range(NSL):
            b = s // (TB // SL)
            off = (s % (TB // SL)) * SL
            o2b = out2_b[b]
            h_sb = hin.tile([128, DFC * SL], BF16, tag="hin", name=f"hin{sfx}")
            for dffc in range(DFC):
                nc.sync.dma_start(h_sb[:, dffc * SL:(dffc + 1) * SL],
                                  h_spill[dffc * 128:(dffc + 1) * 128,
                                          s * SL:(s + 1) * SL])
            for t3 in range(SL // 128):
                toff = off + t3 * 128
                tok0 = b * TB + toff
                rt = [rps.tile([128, 512], F32, tag="r3", name=f"r3{sfx}{half}")
                      for half in range(2)]
                for half in range(2):
                    for dffc in range(DFC):
                        _mmb(nc, rt[half][:],
                             h_sb[:, dffc * SL + t3 * 128:dffc * SL + t3 * 128 + 128],
                             w2_sb[:, dffc * D + half * 512:dffc * D + half * 512 + 512],
                             dffc == 0, dffc == DFC - 1, True)
                # transpose out2 block (bf16) for the token-major residual
                o2t = ops.tile([128, D], BF16, tag="o2t", name=f"o2t{sfx}")
                for kc in range(DC):
                    nc.tensor.matmul(o2t[:, kc * 128:(kc + 1) * 128],
                                     o2b[:, kc * TB + toff:kc * TB + toff + 128],
                                     identb[:], start=True, stop=True,
                                     is_transpose=True, skip_group_check=True)
                # residual + b2 in one pass
                o2ts = o3p.tile([128, D], F32, tag="o2ts", name=f"o2ts{sfx}")
                nc.vector.tensor_tensor(o2ts[:], o2t[:], b2bc[:], AluOpType.add)
                r3 = r3p.tile([128, D], F32, tag="r3s", name=f"r3s{sfx}")
                for half in range(2):
                    nc.vector.tensor_tensor(r3[:, half * 512:(half + 1) * 512],
                                            rt[half][:],
                                            o2ts[:, half * 512:(half + 1) * 512],
                                            AluOpType.add)
                # token-major LN3: stats along the free (feature) dim
                Sc = stp.tile([128, 1], F32, tag="st3", name=f"Sc{sfx}")
                nc.vector.tensor_reduce(Sc[:], r3[:], AX.X, AluOpType.add)
                mneg = stp.tile([128, 1], F32, tag="st3", name=f"mneg3{sfx}")
                nc.vector.tensor_scalar(mneg[:], Sc[:], -1.0 / D, None,
                                        AluOpType.mult)
                sq = sqp.tile([128, D], F32R, tag="sq3", name=f"sq3{sfx}")
                nc.scalar.activation(sq[:], r3[:], AF.Square)
                SSc = stp.tile([128, 1], F32, tag="st3", name=f"SSc{sfx}")
                nc.vector.tensor_reduce(SSc[:], sq[:], AX.X, AluOpType.add)
                m2 = stp.tile([128, 1], F32, tag="st3", name=f"m23{sfx}")
                nc.vector.tensor_tensor(m2[:], mneg[:], mneg[:], AluOpType.mult)
                ve = stp.tile([128, 1], F32, tag="st3", name=f"ve3{sfx}")
                nc.vector.scalar_tensor_tensor(ve[:], in0=SSc[:], scalar=1.0 / D,
                                               in1=m2[:], op0=AluOpType.mult,
                                               op1=AluOpType.subtract)
                sqv = stp.tile([128, 1], F32, tag="st3", name=f"sqv3{sfx}")
                nc.scalar.activation(sqv[:], ve[:], AF.Sqrt, bias=eps_c[:])
                a_ = stp.tile([128, 1], F32, tag="st3", name=f"a3{sfx}")
                nc.vector.reciprocal(a_[:], sqv[:])
                n1 = stp.tile([128, 1], F32, tag="st3", name=f"n13{sfx}")
                nc.vector.tensor_tensor(n1[:], a_[:], a_[:], AluOpType.mult)
                n2 = stp.tile([128, 1], F32, tag="st3", name=f"n23{sfx}")
                nc.vector.scalar_tensor_tensor(n2[:], in0=ve[:], scalar=EPS_LN,
                                               in1=n1[:], op0=AluOpType.add,
                                               op1=AluOpType.mult)
                nc.vector.tensor_scalar(n2[:], n2[:], -0.5, 1.5,
                                        AluOpType.mult, AluOpType.add)
                nc.vector.tensor_tensor(a_[:], a_[:], n2[:], AluOpType.mult)
                # apply: out = ((r3 - m) * rstd) * g3 + be3
                o3 = o3p.tile([128, D], F32, tag="o3", name=f"o3{sfx}")
                nc.vector.tensor_scalar(o3[:], r3[:], mneg[:], a_[:],
                                        AluOpType.add, AluOpType.mult)
                nc.gpsimd.tensor_tensor(o3[:], o3[:], g3bc[:], AluOpType.mult)
                nc.gpsimd.tensor_tensor(o3[:], o3[:], be3bc[:], AluOpType.add)
                nc.sync.dma_start(out_d[tok0:tok0 + 128, :], o3[:])
    w2scope.close()
    o2p.close()


def _host_prep(inputs):
    """Build per-core in_maps from full inputs."""
    f32 = np.float32
    bf16 = ml_dtypes.bfloat16
    x = np.asarray(inputs['x'], f32)
    enc = np.asarray(inputs['enc_output'], f32)

    def fold(Wq, P):
        # [D, H, DH] x [M, DH] -> [D, H*M]
        w = np.einsum('dhk,mk->dhm', np.asarray(Wq, f32), np.asarray(P, f32))
        return np.ascontiguousarray(w.reshape(D, HM) / np.sqrt(M)).astype(bf16)

    e16T = np.zeros((HM, H), bf16)
    e16 = np.zeros((H, HM), bf16)
    for h in range(H):
        e16T[h * M:(h + 1) * M, h] = 1.0
        e16[h, h * M:(h + 1) * M] = 1.0

    gbe = np.zeros((128, 4 * DC), f32)
    for i, nm in enumerate(['g1', 'be1', 'g2', 'be2']):
        gbe[:, i * DC:(i + 1) * DC] = np.asarray(inputs[nm], f32).reshape(DC, 128).T

    shared = {
        'wqe1': fold(inputs['Wq1'], inputs['P1']),
        'wke1': fold(inputs['Wk1'], inputs['P1']),
        'wqe2': fold(inputs['Wq2'], inputs['P2']),
        'wke2': fold(inputs['Wk2'], inputs['P2']),
        'wv1': np.asarray(inputs['Wv1'], f32).reshape(D, D).astype(bf16),
        'wo1': np.asarray(inputs['Wo1'], f32).reshape(D, D).astype(bf16),
        'wv2': np.asarray(inputs['Wv2'], f32).reshape(D, D).astype(bf16),
        'wo2': np.asarray(inputs['Wo2'], f32).reshape(D, D).astype(bf16),
        'w1': np.asarray(inputs['W1'], f32).astype(bf16),
        'w2': np.asarray(inputs['W2'], f32).astype(bf16),
        'e16T': e16T, 'e16': e16,
        'gbe': gbe,
        'b1c': np.ascontiguousarray(np.asarray(inputs['b1'], f32).reshape(DFC, 128).T),
        'b2r': np.asarray(inputs['b2'], f32).reshape(1, D),
        'g3r': np.asarray(inputs['g3'], f32).reshape(1, D),
        'be3r': np.asarray(inputs['be3'], f32).reshape(1, D),
        'identb': np.eye(128, dtype=bf16),
        'ones_col': np.ones((128, 8), bf16),
        'ones_row': np.ones((1, 128), f32),
        'sbe1': np.asarray(inputs['be1'], f32).sum().reshape(1, 1),
    }

    in_maps = []
    for i in range(NCORES):
        sl = slice(i * LSH, (i + 1) * LSH)
        m = dict(shared)
        xs = x[:, sl, :]
        m['xT'] = np.ascontiguousarray(
            xs.transpose(2, 0, 1).reshape(D, T)).astype(bf16)
        m['encT'] = np.ascontiguousarray(
            enc[:, sl, :].transpose(2, 0, 1).reshape(D, T)).astype(bf16)
        m['xtm'] = np.ascontiguousarray(xs.reshape(T, D)).astype(bf16)
        m['sx'] = np.ascontiguousarray(xs.sum(-1).reshape(1, T)).astype(f32)
        in_maps.append(m)
    return in_maps


def kernel(**inputs) -> np.ndarray:
    if 'nc' not in _cache:
        _cache['nc'] = build_program()
    nc = _cache['nc']
    in_maps = _host_prep(inputs)
    res = run_bass_kernel_spmd(nc, in_maps, core_ids=list(range(NCORES)))
    out = np.empty((B, L, D), np.float32)
    for i in range(NCORES):
        o = res.results[i]['out']  # [T, D] token-major
        out[:, i * LSH:(i + 1) * LSH, :] = o.reshape(B, LSH, D)
    return out


if __name__ == '__main__':
    print("building program...")
    build_program()
    print("OK")


# revision 24
# speedup vs baseline: 1.2604x; 1.1269x over previous
"""Trainium2 Bass kernel for nn_DecoderLayer (Performer/FAVOR+ decoder layer).

v3: PE-cycle-lean version of the folded-projection FAVOR+ kernel.

Key ideas on top of v2:
- All matmuls run bf16 (f32r streams ~1.45 cyc/row vs bf16's 1.0).
- kv/kvW reconstruction exploits the per-head block-diagonal mask: instead of
  the full DxD product (masked afterwards, wasting 15/16 of the work), each
  128-row d-chunk kc holds exactly heads (2kc, 2kc+1), so kv is computed only
  for those heads' feature columns (N=64 per chunk) and kvW needs one
  [128,64]x[128,512] matmul per (chunk, half) with a zero-padded lhsT.
- LN1/LN2 token-sum S comes from one matmul with the kvw row-sum vector
  (sum_d po = (sum_d kvw)^T qps) plus a host-computed residual feature-sum
  (attn1) or the be1-sum constant (attn2), replacing 8 accumulated matmuls.
- FFN biases fold into existing elementwise ops (b2 via a broadcast tile,
  b1 via the ELU's activation bias) - no ones-row bias matmuls.
- attn1's ST statistic uses a host-provided token-major x copy instead of
  32 PE transposes per batch.
- Elementwise work in the attention epilogue is spread across DVE /
  Activation / GpSimd so the tensor engine stays busy (p-state residency).

Sharding: sequence (L) split across 8 cores; the per-batch global statistics
ST = [X^T kp | kp-sums] (~2MB total for B=4) are AllReduced twice.
Residual stream is feature-major; the final LN3 runs token-major so the
output DMAs out in natural [T, D] layout (no host post-transpose).
"""
import sys
import os

sys.path.insert(0, '/opt/trn_rl_repo')

import numpy as np
import ml_dtypes
from contextlib import ExitStack

from concourse import bass, bacc, tile
import concourse.mybir as mybir
from concourse.bass_utils import run_bass_kernel_spmd
from concourse.alu_op_type import AluOpType

F32 = mybir.dt.float32
F32R = mybir.dt.float32r
BF16 = mybir.dt.bfloat16
AF = mybir.ActivationFunctionType
AX = mybir.AxisListType

B, L, D, H, DH, M, DFF = 4, 4096, 1024, 16, 64, 8, 4096
NCORES = 8
LSH = L // NCORES          # 512 tokens of L per core
T = B * LSH                # 2048 tokens per core
TB = LSH                   # tokens per batch element per core (512)
NTS = TB // 128            # 4 x 128-token blocks per batch
DC = D // 128              # 8 d-chunks
DFC = DFF // 128           # 32 dff-chunks
HM = H * M                 # 128 random features total
HPC = 2                    # heads per 128-row d-chunk (128 // DH)
EPS_LN = 1e-6
STAB = 0.001
# allreduce width: block-diag kv (8 chunks x 4 batches x 16 features) + ksums.
# kv = mask(Wv^T ST) is linear in ST, so Wv applies to the LOCAL ST before
# the reduce and only the 8x-smaller kv statistic crosses cores.
ARW = DC * B * 2 * M + 4   # 516

_cache = {}


def _mm(nc, out, lhsT, rhs, start, stop, skip=False):
    nc.tensor.matmul(out, lhsT.bitcast(F32R), rhs.bitcast(F32R),
                     start=start, stop=stop, skip_group_check=skip)


def _mmb(nc, out, lhsT, rhs, start, stop, skip=False):
    nc.tensor.matmul(out, lhsT, rhs, start=start, stop=stop,
                     skip_group_check=skip)


def build_program(loop=1, ablate=None):
    nc = bacc.Bacc("TRN2", target_bir_lowering=False, debug=False,
                   num_devices=NCORES)

    def din(name, shape, dt=F32R):
        return nc.dram_tensor(name, shape, dt, kind="ExternalInput").ap()

    tensors = dict(
        xT=din("xT", [D, T], BF16),
        encT=din("encT", [D, T], BF16),
        xtm=din("xtm", [T, D], BF16),
        wqe1=din("wqe1", [D, HM], BF16), wke1=din("wke1", [D, HM], BF16),
        wqe2=din("wqe2", [D, HM], BF16), wke2=din("wke2", [D, HM], BF16),
        wv1=din("wv1", [D, D], BF16), wo1=din("wo1", [D, D], BF16),
        wv2=din("wv2", [D, D], BF16), wo2=din("wo2", [D, D], BF16),
        w1=din("w1", [D, DFF], BF16), w2=din("w2", [DFF, D], BF16),
        e16T_d=din("e16T", [HM, H], BF16), e16_d=din("e16", [H, HM], BF16),
        gbe_d=din("gbe", [128, 4 * DC], F32),
        b1c_d=din("b1c", [128, DFC], F32),
        b2r_d=din("b2r", [1, D]),
        g3r_d=din("g3r", [1, D]), be3r_d=din("be3r", [1, D]),
        identb_d=din("identb", [128, 128], BF16),
        ones_col_d=din("ones_col", [128, 8], BF16),
        ones_row_d=din("ones_row", [1, 128]),
        sx_d=din("sx", [1, T], F32),
        sbe1_d=din("sbe1", [1, 1], F32),
    )

    out_d = nc.dram_tensor("out", [T, D], F32, kind="ExternalOutput").ap()

    with nc.allow_low_precision(reason="bf16 matmul inputs"), \
         tile.TileContext(nc) as tc, ExitStack() as top:
        dram = top.enter_context(tc.tile_pool(name="dram", bufs=1, space="DRAM"))

        const = top.enter_context(tc.tile_pool(name="const", bufs=1))
        c = {}
        c['e16T'] = const.tile([HM, H], BF16, name="e16T")
        nc.sync.dma_start(c['e16T'][:], tensors['e16T_d'][:])
        c['e16'] = const.tile([H, HM], BF16, name="e16")
        nc.sync.dma_start(c['e16'][:], tensors['e16_d'][:])
        c['gbe'] = const.tile([128, 4 * DC], F32, name="gbe")
        nc.sync.dma_start(c['gbe'][:], tensors['gbe_d'][:])
        c['b1c'] = const.tile([128, DFC], F32, name="b1c")
        nc.sync.dma_start(c['b1c'][:], tensors['b1c_d'][:])
        c['identb'] = const.tile([128, 128], BF16, name="identb")
        nc.sync.dma_start(c['identb'][:], tensors['identb_d'][:])
        c['ones_col'] = const.tile([128, 8], BF16, name="ones_col")
        nc.sync.dma_start(c['ones_col'][:], tensors['ones_col_d'][:])
        c['ones_row'] = const.tile([1, 128], F32R, name="ones_row")
        nc.sync.dma_start(c['ones_row'][:], tensors['ones_row_d'][:])
        c['sbe1'] = const.tile([1, 1], F32, name="sbe1")
        nc.sync.dma_start(c['sbe1'][:], tensors['sbe1_d'][:])
        c['eps_t'] = const.tile([1, 1], F32, name="eps_t")
        nc.vector.memset(c['eps_t'][:], EPS_LN)
        c['eps_c'] = const.tile([128, 1], F32, name="eps_c")
        nc.vector.memset(c['eps_c'][:], EPS_LN)
        c['g3bc'] = const.tile([128, D], F32R, name="g3bc")
        c['be3bc'] = const.tile([128, D], F32R, name="be3bc")
        c['b2bc'] = const.tile([128, D], F32R, name="b2bc")
        # row-constant sources live in a transient pool so their [1, D]
        # allocations don't hold per-partition SBUF for the whole kernel
        with tc.tile_pool(name="rowc", bufs=1) as rowp, \
             tc.tile_pool(name="bc_ps", bufs=2, space="PSUM") as bcp:
            for srcd, dstk in ((tensors['b2r_d'], 'b2bc'),
                               (tensors['g3r_d'], 'g3bc'),
                               (tensors['be3r_d'], 'be3bc')):
                src = rowp.tile([1, D], F32R, tag="rowc", name=f"r{dstk}")
                nc.sync.dma_start(src[:], srcd[:])
                for half in range(2):
                    pg = bcp.tile([128, D // 2], F32, tag="bc", name="pbc")
                    _mm(nc, pg[:], c['ones_row'][:],
                        src[:, half * 512:(half + 1) * 512], True, True)
                    nc.any.tensor_copy(c[dstk][:, half * 512:(half + 1) * 512],
                                       pg[:])

        for it in range(loop):
            build_iter(nc, tc, f"i{it}" if loop > 1 else "", tensors, c,
                       dram, out_d, ablate=ablate)

    nc.compile()
    return nc


def build_iter(nc, tc, sfx, tensors, c, dram, out_d, ablate=None):
    e16T, e16 = c['e16T'], c['e16']
    gbe, b1c = c['gbe'], c['b1c']
    identb = c['identb']
    ones_col, ones_row = c['ones_col'], c['ones_row']
    eps_t, g3bc, be3bc, b2bc = c['eps_t'], c['g3bc'], c['be3bc'], c['b2bc']
    eps_c = c['eps_c']

    arin1 = dram.tile([128, ARW], F32, name=f"arin1{sfx}")
    arout1 = dram.tile([128, ARW], F32, addr_space="Shared", name=f"arout1{sfx}")
    arin2 = dram.tile([128, ARW], F32, name=f"arin2{sfx}")
    arout2 = dram.tile([128, ARW], F32, addr_space="Shared", name=f"arout2{sfx}")

    def gslice(i):
        return gbe[:, 2 * i * DC:(2 * i + 1) * DC]

    def beslice(i):
        return gbe[:, (2 * i + 1) * DC:(2 * i + 2) * DC]

    def load_wide(pool, src_dram, ncols, name, dt=BF16):
        nchunk = src_dram.shape[0] // 128
        t_ = pool.tile([128, nchunk * ncols], dt, name=name)
        for kc in range(nchunk):
            nc.sync.dma_start(t_[:, kc * ncols:(kc + 1) * ncols],
                              src_dram[kc * 128:(kc + 1) * 128, :])
        return t_

    def feat_phase(inp_b, wqe, wke, qp_fm, st_b, ks_loc, xtm_d, suffix):
        """qp/kp features + local ST statistic (bf16) + local ksums.

        xtm_d: DRAM [T, D] token-major input for the ST lhsT (attn1), or None
        to transpose inp_b chunks on the PE (attn2).
        """
        with ExitStack() as ph:
            work = ph.enter_context(tc.tile_pool(name=f"ftw{suffix}", bufs=2))
            kptp = ph.enter_context(tc.tile_pool(name=f"kptp{suffix}", bufs=2))
            pqk = ph.enter_context(tc.tile_pool(name=f"pqk{suffix}", bufs=2, space="PSUM"))
            tpp = ph.enter_context(tc.tile_pool(name=f"tpp{suffix}", bufs=4, space="PSUM"))
            stps = ph.enter_context(tc.tile_pool(name=f"stps{suffix}", bufs=2, space="PSUM"))
            if xtm_d is not None:
                xtp = ph.enter_context(tc.tile_pool(name=f"xtp{suffix}", bufs=2))

            for b in range(B):
                xb = inp_b[b]
                if xtm_d is not None:
                    xtm_b = xtp.tile([128, NTS * D], BF16, tag="xtm",
                                     name=f"xtm{suffix}")
                    for ts in range(NTS):
                        nc.sync.dma_start(
                            xtm_b[:, ts * D:(ts + 1) * D],
                            xtm_d[b * TB + ts * 128:b * TB + ts * 128 + 128, :])
                if qp_fm is not None:
                    pq = pqk.tile([128, TB], F32, tag="pqk", name=f"pq{suffix}")
                    for kc in range(DC):
                        _mmb(nc, pq[:], wqe[:, kc * HM:(kc + 1) * HM],
                             xb[:, kc * TB:(kc + 1) * TB], kc == 0, kc == DC - 1)
                    nc.vector.tensor_scalar(qp_fm[:, b * TB:(b + 1) * TB], pq[:],
                                            0.0, STAB, AluOpType.max, AluOpType.add)
                pk = pqk.tile([128, TB], F32, tag="pqk", name=f"pk{suffix}")
                for kc in range(DC):
                    _mmb(nc, pk[:], wke[:, kc * HM:(kc + 1) * HM],
                         xb[:, kc * TB:(kc + 1) * TB], kc == 0, kc == DC - 1)
                kpf = work.tile([128, TB], BF16, tag="kpf", name=f"kpf{suffix}")
                nc.vector.tensor_scalar(kpf[:], pk[:], 0.0, STAB,
                                        AluOpType.max, AluOpType.add)
                # local k-feature sum for z (per-batch column)
                nc.vector.tensor_reduce(ks_loc[:, b:b + 1],
                                        kpf[:], AX.X, AluOpType.add)
                # kp token-major [128 tok, HM] per 128-token block
                kptm = kptp.tile([128, NTS * 128], BF16, tag="kptm",
                                 name=f"kptm{suffix}")
                for ts in range(NTS):
                    ptt = tpp.tile([128, 128], BF16, tag="tp", name=f"ptt{suffix}")
                    nc.tensor.transpose(ptt[:], kpf[:, ts * 128:(ts + 1) * 128],
                                        identb[:])
                    nc.any.tensor_copy(kptm[:, ts * 128:(ts + 1) * 128], ptt[:])
                # ST[d, hm] += x[l, d]*kp[l, hm] with token-major lhsT
                stA = stps.tile([128, 512], F32, tag="st", name=f"stA{suffix}")
                stB = stps.tile([128, 512], F32, tag="st", name=f"stB{suffix}")
                for kc in range(DC):
                    stt = stA if kc < 4 else stB
                    col = (kc % 4) * 128
                    for ts in range(NTS):
                        if xtm_d is not None:
                            lhsT = xtm_b[:, ts * D + kc * 128:ts * D + kc * 128 + 128]
                        else:
                            xtt = tpp.tile([128, 128], BF16, tag="tp",
                                           name=f"xtt{suffix}")
                            nc.tensor.transpose(
                                xtt[:], xb[:, kc * TB + ts * 128:kc * TB + ts * 128 + 128],
                                identb[:])
                            lhsT = work.tile([128, 128], BF16, tag="xts",
                                             name=f"xts{suffix}")
                            nc.any.tensor_copy(lhsT[:], xtt[:])
                            lhsT = lhsT[:]
                        _mmb(nc, stt[:, col:col + 128], lhsT,
                             kptm[:, ts * 128:(ts + 1) * 128],
                             ts == 0, ts == NTS - 1, True)
                for kc in range(DC):
                    stt = stA if kc < 4 else stB
                    col = (kc % 4) * 128
                    nc.any.tensor_copy(
                        st_b[:, kc * 512 + b * 128:kc * 512 + b * 128 + 128],
                        stt[:, col:col + 128])

    def kv_phase(wv_sb, st_b, ks_loc, arin, suffix):
        """Local block-diag kv from local ST -> small AR payload (DRAM)."""
        with ExitStack() as ph:
            arp = ph.enter_context(tc.tile_pool(name=f"arp{suffix}", bufs=1))
            kvps = ph.enter_context(tc.tile_pool(name=f"kvps{suffix}", bufs=2,
                                                 space="PSUM"))
            ar_sb = arp.tile([128, ARW], F32, name=f"ar_sb{suffix}")
            st4 = st_b[:].rearrange("p (dc b hm) -> p dc b hm", dc=DC, b=B)
            for kc in range(DC):
                pkv = kvps.tile([128, B * 2 * M], F32, tag="pkv",
                                name=f"pkv{suffix}")
                for dc in range(DC):
                    _mmb(nc, pkv[:],
                         wv_sb[:, dc * D + kc * 128:dc * D + kc * 128 + 128],
                         st4[:, dc, :, 2 * M * kc:2 * M * kc + 2 * M],
                         dc == 0, dc == DC - 1)
                nc.any.tensor_copy(
                    ar_sb[:, kc * B * 2 * M:(kc + 1) * B * 2 * M], pkv[:])
            nc.any.tensor_copy(ar_sb[:, DC * B * 2 * M:ARW], ks_loc[:])
            nc.sync.dma_start(arin[:], ar_sb[:])

    def attn_out_phase(wo_d, arout, qp_fm, inp_b, out_pool, out_tag,
                       out_dt, ln_idx, suffix, s_resid=None):
        """kvW from the AR'd block-diag kv + o = qps @ kvW + residual + LN.

        s_resid: [1, T] AP of per-token residual feature-sums (attn1), or None
        to use the be1-sum constant (attn2).
        """
        out_b_list = []
        with ExitStack() as ph:
            gps = ph.enter_context(tc.tile_pool(name=f"gps{suffix}", bufs=3, space="PSUM"))
            sps = ph.enter_context(tc.tile_pool(name=f"sps{suffix}", bufs=5, space="PSUM"))
            ksp = ph.enter_context(tc.tile_pool(name=f"ksp{suffix}", bufs=1))
            kvwp = ph.enter_context(tc.tile_pool(name=f"kvwp{suffix}", bufs=1))

            ksums = ksp.tile([128, 4], F32, name=f"ksums{suffix}")
            kvw = kvwp.tile([128, B * D], BF16, name=f"kvw{suffix}")
            # zero-padded stage-2 lhsT: [128, kc, (b, j)] with cross-head
            # blocks zero (row p of chunk kc belongs to head 2kc + p//64;
            # col j<8 is head 2kc, j>=8 head 2kc+1)
            kvz = ksp.tile([128, DC, B, 2 * M], BF16, name=f"kvz{suffix}")
            nc.vector.memset(kvz[:], 0.0)
            with ExitStack() as sg:
                stg = sg.enter_context(tc.tile_pool(name=f"stg{suffix}", bufs=1))
                arf = stg.tile([128, ARW], F32, name=f"arf{suffix}")
                nc.sync.dma_start(arf[:], arout[:])
                nc.any.tensor_copy(ksums[:], arf[:, DC * B * 2 * M:ARW])
                arf4 = arf[:, 0:DC * B * 2 * M].rearrange(
                    "p (kc b j) -> p kc b j", kc=DC, b=B)
                for kc in range(DC):
                    nc.any.tensor_copy(kvz[0:64, kc, :, 0:M],
                                       arf4[0:64, kc, :, 0:M])
                    nc.any.tensor_copy(kvz[64:128, kc, :, M:2 * M],
                                       arf4[64:128, kc, :, M:2 * M])
            # stage 2: kvW rows for chunk kc's heads come only from chunk kc.
            # Scatter into kvw's hm-partition layout crosses 32-partition
            # alignment (compute engines require base%32==0), so the per-batch
            # 16-row moves go through an aligned SBUF staging copy + DMA.
            with ExitStack() as s2:
                wop = s2.enter_context(tc.tile_pool(name=f"wop{suffix}", bufs=1))
                stgp = s2.enter_context(tc.tile_pool(name=f"wsg{suffix}", bufs=4))
                wo_sb = load_wide(wop, wo_d, D, f"wo{suffix}")
                for kc in range(DC):
                    for half in range(2):
                        pw = gps.tile([B * 2 * M, 512], F32, tag="g",
                                      name=f"pw{suffix}")
                        _mmb(nc, pw[:], kvz[:, kc, :, :],
                             wo_sb[:, kc * D + half * 512:kc * D + half * 512 + 512],
                             True, True)
                        stg = stgp.tile([B * 2 * M, 512], BF16, tag="wsg",
                                        name=f"wsg{suffix}")
                        nc.any.tensor_copy(stg[:], pw[:])
                        for b in range(B):
                            nc.sync.dma_start(
                                kvw[2 * M * kc:2 * M * (kc + 1),
                                    b * D + half * 512:b * D + half * 512 + 512],
                                stg[2 * M * b:2 * M * (b + 1), :])

            # stage 3: z, o, residual, LN per batch
            fv = ph.enter_context(tc.tile_pool(name=f"fv{suffix}", bufs=2))
            r1p = ph.enter_context(tc.tile_pool(name=f"r1{suffix}", bufs=1))
            sqp = ph.enter_context(tc.tile_pool(name=f"sq{suffix}", bufs=2))
            stp = ph.enter_context(tc.tile_pool(name=f"stt{suffix}", bufs=1))
            for b in range(B):
                # z = per-head sums of qp * ksums, via ks-scaled e16
                e16ks = fv.tile([HM, H], BF16, tag="e16ks", name=f"e16ks{suffix}")
                nc.vector.tensor_scalar(e16ks[:], e16T[:], ksums[:, b:b + 1],
                                        None, AluOpType.mult)
                zps = sps.tile([H, TB], F32, tag="s", name=f"z{suffix}")
                _mmb(nc, zps[:], e16ks[:], qp_fm[:, b * TB:(b + 1) * TB],
                     True, True)
                rz = fv.tile([H, TB], F32R, tag="rz", name=f"rz{suffix}")
                nc.vector.reciprocal(rz[:], zps[:])
                t1 = fv.tile([H, TB], F32, tag="nt1", name=f"nt1{suffix}")
                nc.vector.tensor_tensor(t1[:], zps[:], rz[:], AluOpType.mult)
                nc.vector.tensor_scalar(t1[:], t1[:], -1.0, 2.0,
                                        AluOpType.mult, AluOpType.add)
                nc.vector.tensor_tensor(rz[:], rz[:], t1[:], AluOpType.mult)
                rzb = fv.tile([H, TB], BF16, tag="rzb", name=f"rzb{suffix}")
                nc.any.tensor_copy(rzb[:], rz[:])
                zbc = sps.tile([128, TB], F32, tag="s", name=f"zbc{suffix}")
                _mmb(nc, zbc[:], e16[:], rzb[:], True, True)
                qps_t = fv.tile([128, TB], BF16, tag="qps", name=f"qps{suffix}")
                nc.vector.tensor_tensor(qps_t[:], qp_fm[:, b * TB:(b + 1) * TB],
                                        zbc[:], AluOpType.mult)

                # S = sum_d r1 via kvw row-sums + residual feature-sums
                wbf = fv.tile([128, 1], F32, tag="wbf", name=f"wbf{suffix}")
                nc.vector.tensor_reduce(wbf[:], kvw[:, b * D:(b + 1) * D],
                                        AX.X, AluOpType.add)
                wbb = fv.tile([128, 1], BF16, tag="wbb", name=f"wbb{suffix}")
                nc.any.tensor_copy(wbb[:], wbf[:])
                Sp = sps.tile([1, TB], F32, tag="s", name=f"S{suffix}")
                _mmb(nc, Sp[:], wbb[:], qps_t[:], True, True, True)
                mneg = stp.tile([1, TB], F32, tag="s0", name=f"mneg{suffix}")[:]
                if s_resid is not None:
                    Ssum = stp.tile([1, TB], F32, tag="s4", name=f"Ssum{suffix}")
                    nc.vector.tensor_tensor(Ssum[:], Sp[:],
                                            s_resid[:, b * TB:(b + 1) * TB],
                                            AluOpType.add)
                    nc.vector.tensor_scalar(mneg, Ssum[:], -1.0 / D, None,
                                            AluOpType.mult)
                else:
                    nc.vector.tensor_scalar(mneg, Sp[:], c['sbe1'][:, 0:1],
                                            -1.0 / D, AluOpType.add,
                                            AluOpType.mult)

                r1 = r1p.tile([128, DC * TB], F32R, tag="r1", name=f"r1{suffix}")
                SSp = sps.tile([1, TB], F32, tag="s", name=f"SS{suffix}")
                for mc in range(DC):
                    po = gps.tile([128, TB], F32, tag="g", name=f"po{suffix}")
                    _mmb(nc, po[:], kvw[:, b * D + mc * 128:b * D + mc * 128 + 128],
                         qps_t[:], True, True)
                    nc.vector.tensor_tensor(r1[:, mc * TB:(mc + 1) * TB], po[:],
                                            inp_b[b][:, mc * TB:(mc + 1) * TB],
                                            AluOpType.add)
                    sq = sqp.tile([128, TB], BF16, tag="sq", name=f"sq{suffix}")
                    nc.scalar.activation(sq[:], r1[:, mc * TB:(mc + 1) * TB],
                                         AF.Square)
                    _mmb(nc, SSp[:], ones_col[:, 0:1], sq[:], mc == 0,
                         mc == DC - 1, True)

                # m2 is dead once ve is computed; n1 reuses its buffer (same
                # tag, bufs=1) to stay inside the SBUF budget
                m2 = stp.tile([1, TB], F32, tag="s1", name=f"m2{suffix}")[:]
                ve = stp.tile([1, TB], F32, tag="s2", name=f"ve{suffix}")[:]
                sqv = stp.tile([1, TB], F32, tag="s3", name=f"sqv{suffix}")[:]
                n1 = stp.tile([1, TB], F32, tag="s1", name=f"n1{suffix}")[:]
                n2 = stp.tile([1, TB], F32, tag="s5", name=f"n2{suffix}")[:]
                a_ = stp.tile([1, TB], F32R, tag="sta", name=f"a{suffix}")
                bb = stp.tile([1, TB], F32R, tag="stb", name=f"bb{suffix}")
                nc.vector.tensor_tensor(m2, mneg, mneg, AluOpType.mult)
                nc.vector.scalar_tensor_tensor(ve, in0=SSp[:], scalar=1.0 / D,
                                               in1=m2, op0=AluOpType.mult,
                                               op1=AluOpType.subtract)
                nc.scalar.activation(sqv, ve, AF.Sqrt, bias=eps_t[:])
                nc.vector.reciprocal(a_, sqv)
                nc.vector.tensor_tensor(n1, a_, a_, AluOpType.mult)
                nc.vector.scalar_tensor_tensor(n2, in0=ve, scalar=EPS_LN,
                                               in1=n1, op0=AluOpType.add,
                                               op1=AluOpType.mult)
                nc.vector.tensor_scalar(n2, n2, -0.5, 1.5,
                                        AluOpType.mult, AluOpType.add)
                nc.vector.tensor_tensor(a_, a_, n2, AluOpType.mult)
                nc.vector.tensor_tensor(bb, mneg, a_, AluOpType.mult)
                abc = sps.tile([128, TB], F32, tag="s", name=f"abc{suffix}")
                _mm(nc, abc[:], ones_row[:], a_[:], True, True)
                bbc = sps.tile([128, TB], F32, tag="s", name=f"bbc{suffix}")
                _mm(nc, bbc[:], ones_row[:], bb[:], True, True)
                # gpsimd cannot read PSUM; stage bbc in SBUF for its adds
                bbcs = fv.tile([128, TB], F32, tag="bbcs", name=f"bbcs{suffix}")
                nc.vector.tensor_copy(bbcs[:], bbc[:])

                ob = out_pool.tile([128, DC * TB], out_dt, tag=out_tag,
                                   name=f"o{out_tag}{suffix}{b}")
                out_b_list.append(ob)
                for mc in range(DC):
                    tpm = sqp.tile([128, TB], F32, tag="sq", name=f"tpm{suffix}")
                    nc.vector.tensor_tensor(tpm[:], r1[:, mc * TB:(mc + 1) * TB],
                                            abc[:], AluOpType.mult)
                    if mc < 5:
                        nc.gpsimd.tensor_tensor(tpm[:], tpm[:], bbcs[:],
                                                AluOpType.add)
                    else:
                        nc.vector.tensor_tensor(tpm[:], tpm[:], bbc[:],
                                                AluOpType.add)
                    if mc < 6:
                        nc.scalar.activation(ob[:, mc * TB:(mc + 1) * TB], tpm[:],
                                             AF.Identity,
                                             bias=beslice(ln_idx)[:, mc:mc + 1],
                                             scale=gslice(ln_idx)[:, mc:mc + 1])
                    else:
                        nc.vector.tensor_scalar(ob[:, mc * TB:(mc + 1) * TB],
                                                tpm[:],
                                                gslice(ln_idx)[:, mc:mc + 1],
                                                beslice(ln_idx)[:, mc:mc + 1],
                                                AluOpType.mult, AluOpType.add)
        return out_b_list

    def allreduce(arin, arout):
        if ablate == 'noar':
            nc.sync.dma_start(arout[:], arin[:])
            return
        nc.gpsimd.collective_compute(
            "AllReduce", AluOpType.add,
            replica_groups=[list(range(NCORES))],
            ins=[arin[:]], outs=[arout[:]])

    # ================= attention 1 + 2 =================
    # o2 pool opens first so it can outlive the mid pools (LIFO release)
    o2p = ExitStack()
    o2pool = o2p.enter_context(tc.tile_pool(name=f"o2p{sfx}", bufs=4))
    if ablate == 'ffn':
        out2_b = []
        for b in range(B):
            ob = o2pool.tile([128, DC * TB], BF16, tag="o2",
                             name=f"oab{sfx}{b}")
            for kc in range(DC):
                nc.sync.dma_start(ob[:, kc * TB:(kc + 1) * TB],
                                  tensors['xT'][kc * 128:(kc + 1) * 128,
                                                b * TB:(b + 1) * TB])
            out2_b.append(ob)
        _build_ffn(nc, tc, sfx, tensors, c, dram, out_d, out2_b, o2p)
        return
    mid = ExitStack()
    resid = mid.enter_context(tc.tile_pool(name=f"resid{sfx}", bufs=5))
    qpp = mid.enter_context(tc.tile_pool(name=f"qpp{sfx}", bufs=2))
    sxp = mid.enter_context(tc.tile_pool(name=f"sxp{sfx}", bufs=1))
    sx_sb = sxp.tile([1, T], F32, name=f"sx{sfx}")
    nc.sync.dma_start(sx_sb[:], tensors['sx_d'][:])

    wqk1 = ExitStack()
    wqkp = wqk1.enter_context(tc.tile_pool(name=f"wqk{sfx}", bufs=1))
    wqe1 = load_wide(wqkp, tensors['wqe1'], HM, f"wqe1{sfx}")
    wke1 = load_wide(wqkp, tensors['wke1'], HM, f"wke1{sfx}")
    wqe2 = load_wide(wqkp, tensors['wqe2'], HM, f"wqe2{sfx}")

    x_b = []
    for b in range(B):
        xb = resid.tile([128, DC * TB], BF16, tag="resid", name=f"x{sfx}{b}")
        for kc in range(DC):
            nc.sync.dma_start(xb[:, kc * TB:(kc + 1) * TB],
                              tensors['xT'][kc * 128:(kc + 1) * 128,
                                            b * TB:(b + 1) * TB])
        x_b.append(xb)

    kv1scope = ExitStack()
    kv1p = kv1scope.enter_context(tc.tile_pool(name=f"kv1p{sfx}", bufs=1))
    wv1_sb = load_wide(kv1p, tensors['wv1'], D, f"wv1{sfx}")
    st_b1 = kv1p.tile([128, DC * 512], BF16, name=f"stb1{sfx}")
    ks1 = kv1p.tile([128, B], F32, name=f"ks1{sfx}")
    qp1 = qpp.tile([HM, T], BF16, tag="qp", name=f"qp1{sfx}")
    feat_phase(x_b, wqe1, wke1, qp1, st_b1, ks1, tensors['xtm'], f"a1{sfx}")
    kv_phase(wv1_sb, st_b1, ks1, arin1, f"a1{sfx}")
    kv1scope.close()
    allreduce(arin1, arout1)

    # overlap with AR1: qp2 from enc_output (streamed)
    qp2 = qpp.tile([HM, T], BF16, tag="qp", name=f"qp2{sfx}")
    with ExitStack() as ph:
        ep = ph.enter_context(tc.tile_pool(name=f"encp{sfx}", bufs=2))
        pqs = ph.enter_context(tc.tile_pool(name=f"pq2{sfx}", bufs=2, space="PSUM"))
        for b in range(B):
            eb = ep.tile([128, DC * TB], BF16, tag="enc", name=f"enc{sfx}")
            for kc in range(DC):
                nc.sync.dma_start(eb[:, kc * TB:(kc + 1) * TB],
                                  tensors['encT'][kc * 128:(kc + 1) * 128,
                                                  b * TB:(b + 1) * TB])
            pq = pqs.tile([128, TB], F32, tag="pq2", name=f"pq2{sfx}")
            for kc in range(DC):
                _mmb(nc, pq[:], wqe2[:, kc * HM:(kc + 1) * HM],
                     eb[:, kc * TB:(kc + 1) * TB], kc == 0, kc == DC - 1)
            nc.vector.tensor_scalar(qp2[:, b * TB:(b + 1) * TB], pq[:],
                                    0.0, STAB, AluOpType.max, AluOpType.add)
    wqk1.close()

    out1_b = attn_out_phase(tensors['wo1'], arout1, qp1, x_b,
                            resid, "resid", BF16, 0, f"a1{sfx}",
                            s_resid=sx_sb)

    wqk2 = ExitStack()
    wqkp2 = wqk2.enter_context(tc.tile_pool(name=f"wk2p{sfx}", bufs=1))
    wke2 = load_wide(wqkp2, tensors['wke2'], HM, f"wke2{sfx}")
    wv2_sb = load_wide(wqkp2, tensors['wv2'], D, f"wv2{sfx}")
    st_b2 = wqkp2.tile([128, DC * 512], BF16, name=f"stb2{sfx}")
    ks2 = wqkp2.tile([128, B], F32, name=f"ks2{sfx}")
    feat_phase(out1_b, None, wke2, None, st_b2, ks2, None, f"a2{sfx}")
    kv_phase(wv2_sb, st_b2, ks2, arin2, f"a2{sfx}")
    allreduce(arin2, arout2)
    wqk2.close()

    out2_b = attn_out_phase(tensors['wo2'], arout2, qp2,
                            out1_b, o2pool, "o2", BF16, 1, f"a2{sfx}")
    mid.close()

    if ablate == 'attn':
        with tc.tile_pool(name=f"ao{sfx}", bufs=1) as aop:
            zt = aop.tile([128, D], F32, name=f"zt{sfx}")
            nc.vector.memset(zt[:], 0.0)
            nc.sync.dma_start(out_d[0:128, :], zt[:])
        o2p.close()
        return
    _build_ffn(nc, tc, sfx, tensors, c, dram, out_d, out2_b, o2p)


def _build_ffn(nc, tc, sfx, tensors, c, dram, out_d, out2_b, o2p):
    identb = c['identb']
    b1c = c['b1c']
    g3bc, be3bc, b2bc = c['g3bc'], c['be3bc'], c['b2bc']
    eps_c = c['eps_c']
    h_spill = dram.tile([DFF, T], BF16, name=f"hspill{sfx}")

    def load_wide(pool, src_dram, ncols, name, dt=BF16):
        nchunk = src_dram.shape[0] // 128
        t_ = pool.tile([128, nchunk * ncols], dt, name=name)
        for kc in range(nchunk):
            nc.sync.dma_start(t_[:, kc * ncols:(kc + 1) * ncols],
                              src_dram[kc * 128:(kc + 1) * 128, :])
        return t_

    # ================= FFN (h = elu(out2 @ W1 + b1), spilled bf16) ==========
    SL = 512
    NSL = T // SL
    # W2 pool opens (and its DMA issues) before FFN1 so the 8MB load overlaps
    # the h-stage compute instead of serializing the FFN1->FFN2 transition.
    w2scope = ExitStack()
    w2pool = w2scope.enter_context(tc.tile_pool(name=f"w2p{sfx}", bufs=1))
    w2_sb = load_wide(w2pool, tensors['w2'], D, f"w2{sfx}")
    with ExitStack() as ph:
        wp = ph.enter_context(tc.tile_pool(name=f"w1p{sfx}", bufs=1))
        hstg = ph.enter_context(tc.tile_pool(name=f"hstg{sfx}", bufs=3))
        ep_ = ph.enter_context(tc.tile_pool(name=f"ep{sfx}", bufs=3))
        hps = ph.enter_context(tc.tile_pool(name=f"hps{sfx}", bufs=8, space="PSUM"))
        w1_sb = load_wide(wp, tensors['w1'], DFF, f"w1{sfx}")
        for s in range(NSL):
            b = s // (TB // SL)
            off = (s % (TB // SL)) * SL
            o2b = out2_b[b]
            for dffc in range(DFC):
                hps_t = hps.tile([128, SL], F32, tag="h", name=f"hps{sfx}")
                for kc in range(DC):
                    _mmb(nc, hps_t[:],
                         w1_sb[:, kc * DFF + dffc * 128:kc * DFF + dffc * 128 + 128],
                         o2b[:, kc * TB + off:kc * TB + off + SL],
                         kc == 0, kc == DC - 1)
                # ELU(u + b1) = min(exp(u+b1) - 1, max(u+b1, 0))
                e_ = ep_.tile([128, SL], BF16, tag="e", name=f"e{sfx}")
                nc.scalar.activation(e_[:], hps_t[:], AF.Exp,
                                     bias=b1c[:, dffc:dffc + 1])
                t_ = ep_.tile([128, SL], BF16, tag="t", name=f"t{sfx}")
                nc.vector.tensor_scalar(t_[:], hps_t[:], b1c[:, dffc:dffc + 1],
                                        0.0, AluOpType.add, AluOpType.max)
                h_ = hstg.tile([128, SL], BF16, tag="hsb", name=f"h{sfx}")
                nc.vector.scalar_tensor_tensor(h_[:], in0=e_[:], scalar=1.0,
                                               in1=t_[:], op0=AluOpType.subtract,
                                               op1=AluOpType.min)
                nc.sync.dma_start(
                    h_spill[dffc * 128:(dffc + 1) * 128,
                            s * SL:(s + 1) * SL], h_[:])

    # ============ r3 = h @ W2 + b2 + out2 ; token-major LN3 -> out ==========
    with ExitStack() as ph:
        hin = ph.enter_context(tc.tile_pool(name=f"hin{sfx}", bufs=2))
        r3p = ph.enter_context(tc.tile_pool(name=f"r3p{sfx}", bufs=2))
        o3p = ph.enter_context(tc.tile_pool(name=f"o3p{sfx}", bufs=2))
        sqp = ph.enter_context(tc.tile_pool(name=f"sq3{sfx}", bufs=1))
        stp = ph.enter_context(tc.tile_pool(name=f"st3{sfx}", bufs=8))
        rps = ph.enter_context(tc.tile_pool(name=f"rps{sfx}", bufs=4, space="PSUM"))
        ops = ph.enter_context(tc.tile_pool(name=f"ops{sfx}", bufs=2, space="PSUM"))

        for s in range(NSL):
            b = s // (TB // SL)
            off = (s % (TB // SL)) * SL
            o2b = out2_b[b]
            h_sb = hin.tile([128, DFC * SL], BF16, tag="hin", name=f"hin{sfx}")
            for dffc in range(DFC):
                nc.sync.dma_start(h_sb[:, dffc * SL:(dffc + 1) * SL],
                                  h_spill[dffc * 128:(dffc + 1) * 128,
                                          s * SL:(s + 1) * SL])
            for t3 in range(SL // 128):
                toff = off + t3 * 128
                tok0 = b * TB + toff
                rt = [rps.tile([128, 512], F32, tag="r3", name=f"r3{sfx}{half}")
                      for half in range(2)]
                for half in range(2):
                    for dffc in range(DFC):
                        _mmb(nc, rt[half][:],
                             h_sb[:, dffc * SL + t3 * 128:dffc * SL + t3 * 128 + 128],
                             w2_sb[:, dffc * D + half * 512:dffc * D + half * 512 + 512],
                             dffc == 0, dffc == DFC - 1, True)
                # transpose out2 block (bf16) for the token-major residual
                o2t = ops.tile([128, D], BF16, tag="o2t", name=f"o2t{sfx}")
                for kc in range(DC):
                    nc.tensor.matmul(o2t[:, kc * 128:(kc + 1) * 128],
                                     o2b[:, kc * TB + toff:kc * TB + toff + 128],
                                     identb[:], start=True, stop=True,
                                     is_transpose=True, skip_group_check=True)
                # residual + b2 in one pass
                o2ts = o3p.tile([128, D], F32, tag="o2ts", name=f"o2ts{sfx}")
                nc.vector.tensor_tensor(o2ts[:], o2t[:], b2bc[:], AluOpType.add)
                r3 = r3p.tile([128, D], F32, tag="r3s", name=f"r3s{sfx}")
                for half in range(2):
                    nc.vector.tensor_tensor(r3[:, half * 512:(half + 1) * 512],
                                            rt[half][:],
                                            o2ts[:, half * 512:(half + 1) * 512],
                                            AluOpType.add)
                # token-major LN3: stats along the free (feature) dim
                Sc = stp.tile([128, 1], F32, tag="st3", name=f"Sc{sfx}")
                nc.vector.tensor_reduce(Sc[:], r3[:], AX.X, AluOpType.add)
                mneg = stp.tile([128, 1], F32, tag="st3", name=f"mneg3{sfx}")
                nc.vector.tensor_scalar(mneg[:], Sc[:], -1.0 / D, None,
                                        AluOpType.mult)
                sq = sqp.tile([128, D], F32R, tag="sq3", name=f"sq3{sfx}")
                nc.scalar.activation(sq[:], r3[:], AF.Square)
                SSc = stp.tile([128, 1], F32, tag="st3", name=f"SSc{sfx}")
                nc.vector.tensor_reduce(SSc[:], sq[:], AX.X, AluOpType.add)
                m2 = stp.tile([128, 1], F32, tag="st3", name=f"m23{sfx}")
                nc.vector.tensor_tensor(m2[:], mneg[:], mneg[:], AluOpType.mult)
                ve = stp.tile([128, 1], F32, tag="st3", name=f"ve3{sfx}")
                nc.vector.scalar_tensor_tensor(ve[:], in0=SSc[:], scalar=1.0 / D,
                                               in1=m2[:], op0=AluOpType.mult,
                                               op1=AluOpType.subtract)
                sqv = stp.tile([128, 1], F32, tag="st3", name=f"sqv3{sfx}")
                nc.scalar.activation(sqv[:], ve[:], AF.Sqrt, bias=eps_c[:])
                a_ = stp.tile([128, 1], F32, tag="st3", name=f"a3{sfx}")
                nc.vector.reciprocal(a_[:], sqv[:])
                n1 = stp.tile([128, 1], F32, tag="st3", name=f"n13{sfx}")
                nc.vector.tensor_tensor(n1[:], a_[:], a_[:], AluOpType.mult)
                n2 = stp.tile([128, 1], F32, tag="st3", name=f"n23{sfx}")
                nc.vector.scalar_tensor_tensor(n2[:], in0=ve[:], scalar=EPS_LN,
                                               in1=n1[:], op0=AluOpType.add,
                                               op1=AluOpType.mult)
                nc.vector.tensor_scalar(n2[:], n2[:], -0.5, 1.5,
                                        AluOpType.mult, AluOpType.add)
                nc.vector.tensor_tensor(a_[:], a_[:], n2[:], AluOpType.mult)
                # apply: out = ((r3 - m) * rstd) * g3 + be3
                o3 = o3p.tile([128, D], F32, tag="o3", name=f"o3{sfx}")
                nc.vector.tensor_scalar(o3[:], r3[:], mneg[:], a_[:],
                                        AluOpType.add, AluOpType.mult)
                nc.gpsimd.tensor_tensor(o3[:], o3[:], g3bc[:], AluOpType.mult)
                nc.gpsimd.tensor_tensor(o3[:], o3[:], be3bc[:], AluOpType.add)
                nc.sync.dma_start(out_d[tok0:tok0 + 128, :], o3[:])
    w2scope.close()
    o2p.close()


def _host_prep(inputs):
    """Build per-core in_maps from full inputs."""
    f32 = np.float32
    bf16 = ml_dtypes.bfloat16
    x = np.asarray(inputs['x'], f32)
    enc = np.asarray(inputs['enc_output'], f32)

    def fold(Wq, P):
        # [D, H, DH] x [M, DH] -> [D, H*M]
        w = np.einsum('dhk,mk->dhm', np.asarray(Wq, f32), np.asarray(P, f32))
        return np.ascontiguousarray(w.reshape(D, HM) / np.sqrt(M)).astype(bf16)

    e16T = np.zeros((HM, H), bf16)
    e16 = np.zeros((H, HM), bf16)
    for h in range(H):
        e16T[h * M:(h + 1) * M, h] = 1.0
        e16[h, h * M:(h + 1) * M] = 1.0

    gbe = np.zeros((128, 4 * DC), f32)
    for i, nm in enumerate(['g1', 'be1', 'g2', 'be2']):
        gbe[:, i * DC:(i + 1) * DC] = np.asarray(inputs[nm], f32).reshape(DC, 128).T

    shared = {
        'wqe1': fold(inputs['Wq1'], inputs['P1']),
        'wke1': fold(inputs['Wk1'], inputs['P1']),
        'wqe2': fold(inputs['Wq2'], inputs['P2']),
        'wke2': fold(inputs['Wk2'], inputs['P2']),
        'wv1': np.asarray(inputs['Wv1'], f32).reshape(D, D).astype(bf16),
        'wo1': np.asarray(inputs['Wo1'], f32).reshape(D, D).astype(bf16),
        'wv2': np.asarray(inputs['Wv2'], f32).reshape(D, D).astype(bf16),
        'wo2': np.asarray(inputs['Wo2'], f32).reshape(D, D).astype(bf16),
        'w1': np.asarray(inputs['W1'], f32).astype(bf16),
        'w2': np.asarray(inputs['W2'], f32).astype(bf16),
        'e16T': e16T, 'e16': e16,
        'gbe': gbe,
        'b1c': np.ascontiguousarray(np.asarray(inputs['b1'], f32).reshape(DFC, 128).T),
        'b2r': np.asarray(inputs['b2'], f32).reshape(1, D),
        'g3r': np.asarray(inputs['g3'], f32).reshape(1, D),
        'be3r': np.asarray(inputs['be3'], f32).reshape(1, D),
        'identb': np.eye(128, dtype=bf16),
        'ones_col': np.ones((128, 8), bf16),
        'ones_row': np.ones((1, 128), f32),
        'sbe1': np.asarray(inputs['be1'], f32).sum().reshape(1, 1),
    }

    in_maps = []
    for i in range(NCORES):
        sl = slice(i * LSH, (i + 1) * LSH)
        m = dict(shared)
        xs = x[:, sl, :]
        m['xT'] = np.ascontiguousarray(
            xs.transpose(2, 0, 1).reshape(D, T)).astype(bf16)
        m['encT'] = np.ascontiguousarray(
            enc[:, sl, :].transpose(2, 0, 1).reshape(D, T)).astype(bf16)
        m['xtm'] = np.ascontiguousarray(xs.reshape(T, D)).astype(bf16)
        m['sx'] = np.ascontiguousarray(xs.sum(-1).reshape(1, T)).astype(f32)
        in_maps.append(m)
    return in_maps


def kernel(**inputs) -> np.ndarray:
    if 'nc' not in _cache:
        _cache['nc'] = build_program()
    nc = _cache['nc']
    in_maps = _host_prep(inputs)
    res = run_bass_kernel_spmd(nc, in_maps, core_ids=list(range(NCORES)))
    out = np.empty((B, L, D), np.float32)
    for i in range(NCORES):
        o = res.results[i]['out']  # [T, D] token-major
        out[:, i * LSH:(i + 1) * LSH, :] = o.reshape(B, LSH, D)
    return out


if __name__ == '__main__':
    print("building program...")
    build_program()
    print("OK")
